# revision 53
# baseline (speedup 1.0000x reference)
"""Trainium2 Bass kernel for nn_Architecture_11879879540882 (AKT-style
monotonic sparse attention), data-parallel over batch on 8 NeuronCores.

Self-contained: hardcodes shapes B=16,S=512,D=256,H=8,DK=32, shards the batch
2-per-core, runs one Bass graph SPMD via run_bass_kernel_spmd, gathers output.

Algorithm notes (validated vs the jax reference):
 - blocks 1/2: k-projection shares q weights and inputs -> K == Q, so the
   score matrix is SYMMETRIC.
 - masked softmax + cumsum distance statistics collapse into ONE reversed
   masked scan: state=(E+state)*mask -> all suffix sums + masked row total.
 - dist = sqrt(suffix*pos*g^2/r), total = exp(-dist)   [g = -softplus(gamma)]
 - second softmax is UNMASKED (reproduces the reference's non-inplace
   masked_fill bug); the 1e-5 clip on `total` is skipped (~1e-4 error).
 - all LayerNorm gamma/beta fold into downstream weights on host.
 - block3's query is position-independent -> its score rows are broadcasts.

Redesign vs the first working version (2.x speedup targets):
 - TRIANGLE NARROWING: for row-chunk I (128 rows), cols >= (I+1)*128 have
   total==1, so phase-1 exp/scan/stat-write and phase-2 sqrt/exp only cover
   (I+1)*128 cols (62.5% of full).
 - TRANSPOSE-BEFORE-EXP: phase 3 transposes z=s*total (PE), then the ACT
   exp reads the transposed PSUM directly -> P^T lands in SBUF with no
   separate PSUM->SBUF copy instruction.
 - SYMMETRY REUSE: for b1/b2 the strict-upper P^T tiles equal phase-1's
   exp(s) tiles (kept in SBUF); for b3 they are rank-1 (broadcast scores)
   and collapse into tiny [1,x] matmuls + a ones-broadcast matmul.
 - ENGINE REBALANCE: distance-stat writes + att copies + final accumulate
   move to the (otherwise idle) Pool engine; bigA is reused in place for
   sqrt/exp/z (saves ~36KB/partition of SBUF).
"""
import sys
import numpy as np

for _p in ('/opt/trn_rl_repo',):
    if _p not in sys.path:
        sys.path.append(_p)

import ml_dtypes
import concourse.bass as bass
import concourse.bacc as bacc
import concourse.tile as tile
import concourse.mybir as mybir
from concourse.bass_utils import run_bass_kernel_spmd

F32 = mybir.dt.float32
BF16 = mybir.dt.bfloat16
Alu = mybir.AluOpType
Act = mybir.ActivationFunctionType
NPBF = ml_dtypes.bfloat16

B, S, D, H, DK = 16, 512, 256, 8, 32
NCORES = 8
BL = B // NCORES          # local batches per core = 2
PC = BL * 4               # 128-row position chunks per core = 8
LN_EPS = 1e-5

REV = (slice(None), slice(None, None, -1))

# packed-constant layouts: (name, ncols); all [128, ncols] bf16.
PACKA = ([('ident', 128)]
         + [(f'b1_{w}__{k}', 256) for w in ('wq', 'wv', 'wo')
            for k in range(2)]
         + [(f'mpad__{i}', 513) for i in range(4)]
         + [(f'posm__{i}', 512) for i in range(4)])
PACKB = ([(f'b2_{w}__{k}', 256) for w in ('wq', 'wv', 'wo')
          for k in range(2)]
         + [(f'b3_{w}__{k}', 256) for w in ('wk', 'wv', 'wo')
            for k in range(2)]
         + [(f'q03__{k}', 8) for k in range(2)]
         + [(f'keyhT__{k}', 8) for k in range(2)])
PACK32 = [('b3_kbT', 2), ('g2b_b1', 8), ('g2b_b2', 8), ('g2b_b3', 8)]


def _softplus(x):
    return np.logaddexp(0.0, x)


def _host_prep(inp):
    """Parameter preprocessing on host. Returns (consts dict, g2 dict)."""
    p = {k: np.asarray(v, np.float32) for k, v in inp.items()}
    c = {}
    s4 = np.float32(DK ** -0.25)
    bf = lambda x: np.ascontiguousarray(np.asarray(x, np.float32)).astype(NPBF)
    colpack = lambda b: np.ascontiguousarray(
        np.asarray(b, np.float32).reshape(2, 128).T).astype(np.float32)

    for blk in ('b1', 'b2'):
        c[blk + '_wq'] = bf(p[blk + '_qw'] * s4)
        c[blk + '_qbr'] = bf((p[blk + '_qb'] * s4)[None, :])
        c[blk + '_wv'] = bf(p[blk + '_vw'])
        c[blk + '_vbr'] = bf(p[blk + '_vb'][None, :])
        c[blk + '_wo'] = bf(p[blk + '_ow'])
        c[blk + '_obr'] = bf(p[blk + '_ob'][None, :])
    know = p['know'][0, 0]
    q03 = ((know @ p['b3_qw'] + p['b3_qb']) / np.sqrt(DK)).reshape(H, DK)
    Q03 = np.zeros((D, H), np.float32)
    for h in range(H):
        Q03[h * DK:(h + 1) * DK, h] = q03[h]
    c['q03'] = bf(Q03)
    g1, be1 = p['b1_lng'], p['b1_lnb']
    c['b3_wk'] = bf(p['b3_kw'] * g1[:, None])
    c['b3_kbT'] = colpack(p['b3_kb'] + be1 @ p['b3_kw'])
    g2_, be2 = p['b2_lng'], p['b2_lnb']
    c['b3_wv'] = bf(p['b3_vw'] * g2_[:, None])
    c['b3_vbr'] = bf((p['b3_vb'] + be2 @ p['b3_vw'])[None, :])
    c['b3_wo'] = bf(p['b3_ow'])
    c['b3_obr'] = bf((p['b3_ob'] + know)[None, :])
    g3, be3 = p['b3_lng'], p['b3_lnb']
    lvw = np.zeros((H, DK, D), np.float32)
    lvb = np.zeros((H, D), np.float32)
    for h in range(H):
        sl = slice(h * DK, (h + 1) * DK)
        lvw[h] = p['lv_w'] * g3[sl][:, None]
        lvb[h] = p['lv_b'] + be3[sl] @ p['lv_w']
    c['lvw'] = bf(lvw)                            # -> lvw__h [32,256]
    c['lvbr'] = bf(lvb.reshape(1, H * D))         # [1, 2048]
    know_r = know.reshape(H, DK)
    keyh = 1.0 / (1.0 + np.exp(-(know_r @ p['lk_w'] + p['lk_b'])))
    c['keyhT'] = bf(keyh.T)                       # [D, H]

    # padded inclusive mask: mpad[ic][p, j] = (j <= i_p), j in [0, 512];
    # strict mask is the shifted view mpad[:, 1:513].
    i = np.arange(S + 1, dtype=np.int64)
    mpad = np.zeros((4, 128, S + 1), np.float32)
    pos = np.zeros((4, 128, S), np.float32)
    for ic in range(4):
        ii = np.arange(ic * 128, (ic + 1) * 128, dtype=np.int64)[:, None]
        mpad[ic] = (i[None, :] <= ii)
        pos[ic] = np.abs(ii - i[None, :S])
    for blk in ('b1', 'b2', 'b3'):
        g2v = (_softplus(p[blk + '_gam'][:, 0, 0]) ** 2).astype(np.float32)
        c['g2b_' + blk] = np.ascontiguousarray(
            np.broadcast_to(g2v[None, :], (128, H))).astype(np.float32)
    c['mpad'] = bf(mpad)
    c['posm'] = bf(pos)
    c['ident'] = bf(np.eye(128))

    flat = {}
    for name, a in c.items():
        if a.ndim == 2 and a.shape[0] > 128:
            for kc in range(a.shape[0] // 128):
                flat[f"{name}__{kc}"] = np.ascontiguousarray(
                    a[kc * 128:(kc + 1) * 128])
        elif a.ndim == 3:
            for kc in range(a.shape[0]):
                flat[f"{name}__{kc}"] = np.ascontiguousarray(a[kc])
        else:
            flat[name] = a
    # pack the [128, x] bf16 consts into two big arrays (2 DMAs instead of
    # ~30 -- the serial SP DMA-issue time dominated kernel startup).
    packed = {}
    for pname, layout in (('packA', PACKA), ('packB', PACKB)):
        tot = sum(w for _, w in layout)
        arr = np.zeros((128, tot), NPBF)
        off = 0
        for nm, wd in layout:
            arr[:, off:off + wd] = flat.pop(nm)
            off += wd
        packed[pname] = arr
    tot = sum(w for _, w in PACK32)
    arr = np.zeros((128, tot), np.float32)
    off = 0
    for nm, wd in PACK32:
        arr[:, off:off + wd] = flat.pop(nm)
        off += wd
    packed['pack32'] = arr
    flat.update(packed)
    g2 = {blk: [float(v) for v in
                (_softplus(p[blk + '_gam'][:, 0, 0]) ** 2)]
          for blk in ('b1', 'b2', 'b3')}
    return flat, g2


_NPDT = {np.dtype(np.float32): F32, np.dtype(NPBF): BF16}


def _build(consts, g2, reps=1):
    """Builds the per-core Bass graph (BL local batches). reps>1 repeats
    the whole computation on-device (for slope-based timing)."""
    nc = bacc.Bacc("TRN2", target_bir_lowering=False, debug=False)

    x1d = nc.dram_tensor("x1", [BL, S, D], F32, kind="ExternalInput")
    x2d = nc.dram_tensor("x2", [BL, S, D], F32, kind="ExternalInput")
    outd = nc.dram_tensor("out", [BL, S, D], F32, kind="ExternalOutput")
    cd = {name: nc.dram_tensor(name, list(a.shape), _NPDT[a.dtype],
                               kind="ExternalInput")
          for name, a in consts.items()}

    from contextlib import ExitStack
    with tile.TileContext(nc) as tc, ExitStack() as _ps:
        sb = _ps.enter_context(tc.tile_pool(name="const", bufs=1))
        work = _ps.enter_context(tc.tile_pool(name="work", bufs=1))
        sm = _ps.enter_context(tc.tile_pool(name="sm", bufs=4))
        rot = _ps.enter_context(tc.tile_pool(name="rot", bufs=4))
        p1 = _ps.enter_context(tc.tile_pool(name="p1", bufs=2, space="PSUM"))
        pT = _ps.enter_context(tc.tile_pool(name="pT", bufs=2, space="PSUM"))
        pAV = _ps.enter_context(tc.tile_pool(name="pAV", bufs=1, space="PSUM"))
        pO = _ps.enter_context(tc.tile_pool(name="pO", bufs=1, space="PSUM"))

        # ---------- constants (packed: 3 big DMAs + a few stragglers) ----
        C = {}
        for pname, layout, dt in (('packA', PACKA, BF16),
                                  ('packB', PACKB, BF16),
                                  ('pack32', PACK32, F32)):
            tot = sum(wd for _, wd in layout)
            t = sb.tile([128, tot], dt, name="c_" + pname)
            nc.sync.dma_start(t[:], cd[pname][:])
            off = 0
            for nm, wd in layout:
                C[nm] = t[:, off:off + wd]
                off += wd

        def _prio(name):
            for i, k in enumerate(('b1_', 'b2_', 'b3_', 'lvbr', 'lv', 'key')):
                if name.startswith(k):
                    return i
            return 99
        for name in sorted(cd, key=_prio):
            if name in ('packA', 'packB', 'pack32'):
                continue
            ap = cd[name]
            t = sb.tile(list(ap.shape), ap.dtype, name="c_" + name)
            nc.sync.dma_start(t[:], ap[:])
            C[name] = t
        ones1 = sb.tile([1, 512], BF16, name="ones1")
        nc.vector.memset(ones1[:], 1.0)
        ones0 = sb.tile([1, 128], BF16, name="ones0")   # [0,1,1,...]
        nc.vector.memset(ones0[:], 1.0)
        nc.vector.memset(ones0[0:1, 0:1], 0.0)
        epsT = sb.tile([128, 1], F32, name="epsT")
        nc.vector.memset(epsT[:], LN_EPS)
        ident = C['ident']
        mpad = [C[f"mpad__{ic}"] for ic in range(4)]
        posm = [C[f"posm__{ic}"] for ic in range(4)]

        def transpose128(dst, src, cp_engine=None):
            """dst[128,128] SBUF bf16 = src.T via PE + copy."""
            pt = pT.tile([128, 1024], BF16, tag="pT", name="pt_t")
            nc.tensor.transpose(pt[:, 0:128], src, ident[:])
            (cp_engine or nc.vector).tensor_copy(dst, pt[:, 0:128])

        # ---------- input prep: transposed bf16 copies of x1/x2 ----------
        # x DMAs issue from the (idle) Pool queue so they run in parallel
        # with the SP queue's long constants sequence.
        for _rep in range(reps):
          xT = {}

          def prep_x(xi, xd):
              for dc in range(2):
                  xT[(xi, dc)] = work.tile([128, BL * S], BF16, tag="xfrm",
                                           bufs=6, name=f"xT{xi}_{dc}")
              for pc in range(PC):
                  b, ic = divmod(pc, 4)
                  t = sm.tile([128, D], F32, tag="xin", name="xin", bufs=3)
                  nc.gpsimd.dma_start(t[:], xd[b, ic * 128:(ic + 1) * 128, :])
                  tb = sm.tile([128, D], BF16, tag="xbf", name="xbf_t", bufs=2)
                  nc.vector.tensor_copy(tb[:], t[:])
                  for dc in range(2):
                      transpose128(xT[(xi, dc)][:, pc * 128:(pc + 1) * 128],
                                   tb[:, dc * 128:(dc + 1) * 128])

          prep_x(1, x1d)

          # ---------- projections ----------
          def projQ(xTloc, wname, brname, out_name):
              """Head-packed transposed projection QTp [64, H/2*BL*S]:
              head h sits at partition base 32*(h%2),
              cols (h//2)*1024 + b*512 + pos."""
              QTp = work.tile([64, (H // 2) * BL * S], BF16, name=out_name,
                              tag="QTp", bufs=2)
              qbr = C[brname]
              for hp in range(H // 2):
                  for half in range(BL):
                      ps = p1.tile([128, 1024], F32, tag="p1",
                                   name="projQ_ps")
                      for sub in range(2):
                          h = 2 * hp + sub
                          bp = 32 * sub
                          for kc in range(2):
                              nc.tensor.matmul(
                                  ps[bp:bp + 32, 0:512],
                                  C[f"{wname}__{kc}"][:, h * 32:(h + 1) * 32],
                                  xTloc[kc][:, half * 512:(half + 1) * 512],
                                  start=(kc == 0), stop=False)
                          nc.tensor.matmul(ps[bp:bp + 32, 0:512],
                                           qbr[0:1, h * 32:(h + 1) * 32],
                                           ones1[0:1, :], start=False,
                                           stop=True)
                      nc.vector.tensor_copy(
                          QTp[0:64, hp * 1024 + half * 512:
                              hp * 1024 + (half + 1) * 512],
                          ps[0:64, 0:512])
              return QTp

          def projT(xTloc, wname, bTname, out_name):
              """Chunk-transposed projection out[dc][128, BL*S] (for K3T)."""
              out = [work.tile([128, BL * S], BF16, tag="xfrm", bufs=6,
                               name=f"{out_name}_{dc}") for dc in range(2)]
              bT = C[bTname]
              for dc in range(2):
                  for hh in range(BL):
                      ps = p1.tile([128, 1024], F32, tag="p1", name="projT_ps")
                      for kc in range(2):
                          nc.tensor.matmul(
                              ps[:, 0:512],
                              C[f"{wname}__{kc}"][:, dc * 128:(dc + 1) * 128],
                              xTloc[kc][:, hh * 512:(hh + 1) * 512],
                              start=(kc == 0), stop=(kc == 1))
                      nc.scalar.activation(out[dc][:, hh * 512:(hh + 1) * 512],
                                           ps[:, 0:512], Act.Identity,
                                           bias=bT[:, dc:dc + 1], scale=1.0)
              return out

          def projN(xTloc, wname, brname, out_name):
              """Natural projection, head-packed with a ones column:
              out[pc] [128, 8*33]: head h = cols [33h, 33h+32), col 33h+32=1.
              The ones column makes the AV matmul also emit softmax-2 row
              sums."""
              out = [work.tile([128, H * 33], BF16, tag="Vt", bufs=12,
                               name=f"{out_name}_{pc}") for pc in range(PC)]
              br = C[brname]
              for pc in range(PC):
                  ps = pO.tile([128, D], F32, tag="pO", name="projN_ps")
                  for kc in range(2):
                      nc.tensor.matmul(ps[:],
                                       xTloc[kc][:, pc * 128:(pc + 1) * 128],
                                       C[f"{wname}__{kc}"],
                                       start=(kc == 0), stop=False)
                  nc.tensor.matmul(ps[:], ones1[0:1, 0:128], br[:],
                                   start=False, stop=True)
                  ov = out[pc].rearrange("p (h c) -> p h c", c=33)
                  nc.vector.tensor_copy(ov[:, :, 0:32],
                                        ps.rearrange("p (h c) -> p h c",
                                                     c=32))
                  nc.gpsimd.memset(ov[:, :, 32:33], 1.0)
              return out

          # ============== attention block phases ==============
          # state[(b)] = dict with Ep tiles, bigA tiles, etc.

          def qk_ph1(QTp, h, I, b, ps, w):
              """phase-1 scores ps[:, 0:w] for one batch-half."""
              hp, sub = divmod(h, 2)
              bp = 32 * sub
              base = hp * 1024 + b * 512
              nc.tensor.matmul(
                  ps[:, 0:w],
                  QTp[bp:bp + 32, base + I * 128: base + I * 128 + 128],
                  QTp[bp:bp + 32, base: base + w],
                  start=True, stop=True)

          def phase1(blk, QTp, b, st=None, Eb3=None):
              """QK + exp + scans + stat-writes for ONE batch-half b.
              Accumulates into/returns state st: Ep[h][I][b], bigA[b][I]."""
              strict = (blk == 'b3')
              g2b = C['g2b_' + blk]
              if st is None:
                  st = {'Ep': [[[None] * BL for _ in range(4)]
                               for _ in range(H)],
                        'bigA': [[None] * 4 for _ in range(BL)],
                        'strict': strict}
              Ep, bigA = st['Ep'], st['bigA']
              for I in range(4):
                  w = (I + 1) * 128
                  bigA[b][I] = work.tile([128, H * w], BF16,
                                         tag=f"bigA{I}", bufs=2,
                                         name=f"bigA_{blk}_{b}_{I}")
              if strict:
                  # Eb3[h][b] = exp(broadcast score row), reused for all I.
                  for h in range(H):
                      for I in range(4):
                          Ep[h][I][b] = Eb3[h][b]
              for I in range(4):
                  w = (I + 1) * 128
                  maskv = (mpad[I][:, 1:w + 1] if strict
                           else mpad[I][:, 0:w])
                  r8 = sm.tile([128, H], F32, tag="r8", name="r8", bufs=2)
                  Rvs = {}
                  if not strict:
                      for hp in range(H // 2):
                          ps = p1.tile([128, 1024], F32, tag="p1",
                                       name="qk1_ps")
                          for sub in range(2):
                              h = 2 * hp + sub
                              bp = 32 * sub
                              base = hp * 1024 + b * 512
                              nc.tensor.matmul(
                                  ps[:, sub * 512: sub * 512 + w],
                                  QTp[bp:bp + 32,
                                      base + I * 128: base + I * 128 + 128],
                                  QTp[bp:bp + 32, base: base + w],
                                  start=True, stop=True)
                          e = work.tile([128, 2 * w], BF16,
                                        tag=f"Ep{hp}_{I}", bufs=2,
                                        name=f"Ep_{hp}_{I}_{b}")
                          nc.scalar.activation(
                              e.rearrange("p (s c) -> p s c", c=w),
                              ps.rearrange("p (s c) -> p s c",
                                           c=512)[:, :, 0:w], Act.Exp)
                          Ep[2 * hp][I][b] = e[:, 0:w]
                          Ep[2 * hp + 1][I][b] = e[:, w:2 * w]
                  for h in range(H):
                      Rv = sm.tile([128, 512], BF16, tag="Rv",
                                   name="Rv", bufs=8)
                      nc.vector.tensor_tensor_scan(
                          Rv[:, 0:w][REV], Ep[h][I][b][:, 0:w][REV],
                          maskv[REV], 0.0, op0=Alu.add, op1=Alu.mult)
                      nc.vector.tensor_copy(r8[:, h:h + 1], Rv[:, 0:1])
                      Rvs[h] = Rv
                  r8m = sm.tile([128, H], F32, tag="r8m", name="r8m", bufs=2)
                  nc.vector.tensor_scalar(r8m[:], r8[:], 1e-30, None,
                                          op0=Alu.max)
                  rc = sm.tile([128, H], F32, tag="rc", name="rc", bufs=2)
                  nc.vector.reciprocal(rc[:], r8m[:])
                  rgb = sm.tile([128, H], F32, tag="rgb", name="rgb", bufs=2)
                  nc.vector.tensor_tensor(rgb[:], rc[:], g2b[:], op=Alu.mult)
                  bA = bigA[b][I]
                  for h in range(H):
                      # Rv *= g^2/r in place (DVE 4x), then the shifted
                      # suffix*pos product on Pool (tensor_tensor mult is
                      # one of the few opcodes GPSIMD supports on HW).
                      nc.vector.tensor_scalar(Rvs[h][:, 0:w], Rvs[h][:, 0:w],
                                              rgb[:, h:h + 1], None,
                                              op0=Alu.mult)
                      nc.gpsimd.tensor_tensor(
                          bA[:, h * w: h * w + w - 1],
                          posm[I][:, 0:w - 1], Rvs[h][:, 1:w], op=Alu.mult)
                  # col w-1 of every head segment: suffix beyond row end
                  # is empty -> bigA = 0 (gives total == 1 post-exp).
                  bAv = bA.rearrange("p (h c) -> p h c", c=w)
                  nc.gpsimd.memset(bAv[:, :, w - 1:w], 0.0)
              return st

          def mk_token(srcs):
              """[128,1] zero tile data-dependent on srcs (ACT-order fence:
              used as a sqrt bias so the sqrt batch can't start, and hence
              can't interleave table-thrashing, before srcs complete)."""
              tok = sm.tile([128, 1], F32, tag="tok", name="tok", bufs=2)
              if len(srcs) == 1:
                  nc.vector.tensor_scalar(tok[:], srcs[0][:, 0:1], 0.0, None,
                                          op0=Alu.mult)
              else:
                  nc.vector.scalar_tensor_tensor(
                      tok[:], srcs[0][:, 0:1], 0.0, srcs[1][:, 0:1],
                      op0=Alu.mult, op1=Alu.mult)
              return tok

          def phase2_sqrt(st, b, token=None):
              for I in range(4):
                  bA = st['bigA'][b][I]
                  if token is not None:
                      nc.scalar.activation(bA[:], bA[:], Act.Sqrt,
                                           bias=token[:, 0:1], scale=1.0)
                  else:
                      nc.scalar.activation(bA[:], bA[:], Act.Sqrt)

          def phase2_exp(st, b):
              for I in range(4):
                  bA = st['bigA'][b][I]
                  nc.scalar.activation(bA[:], bA[:], Act.Exp, scale=-1.0)

          def phase3(blk, st, QTp, V, b, csbs=None, wsb=None,
                     resid_dram=None, out_name="hout"):
              """z -> transpose -> exp(P^T) -> AV -> Wo -> +resid -> LN stats
              for one batch-half. Returns 4 hout tiles.

              b1/b2: z = s*t in place on bigA, then transpose+exp.
              b3: scores are broadcast rows c_j, so z^T = c_j * t^T with c_j
              PER-PARTITION after the transpose -> fold into the exp's scale
              operand (no z multiply, no broadcast-score tiles at all)."""
              strict = st['strict']
              Ep = st['Ep']
              Wo = [C[blk + '_wo__0'], C[blk + '_wo__1']]
              obr = C[blk + '_obr']
              houts = []
              mvh = sm.tile([128, 8], F32, tag="mvh", name="mvh", bufs=2)
              ybufs = []
              for I in range(4):
                  w = (I + 1) * 128
                  bA = st['bigA'][b][I]
                  # ---- z = s * total, in place on bigA (b1/b2 only) ----
                  if not strict:
                      for hp in range(4):
                          ps3 = p1.tile([128, 1024], F32, tag="p1",
                                        name="qk3_ps")
                          for sub in range(2):
                              h = 2 * hp + sub
                              hq, s2 = divmod(h, 2)
                              bp = 32 * s2
                              base = hq * 1024 + b * 512
                              nc.tensor.matmul(
                                  ps3[:, sub * 512: sub * 512 + w],
                                  QTp[bp:bp + 32,
                                      base + I * 128: base + I * 128 + 128],
                                  QTp[bp:bp + 32, base: base + w],
                                  start=True, stop=True)
                          zv = bA[:, 2 * hp * w: (2 * hp + 2) * w].rearrange(
                              "p (h c) -> p h c", c=w)
                          psv = ps3.rearrange("p (h c) -> p h c",
                                              c=512)[:, :, 0:w]
                          nc.vector.tensor_tensor(zv, psv, zv, op=Alu.mult)
                  # ---- transposed P^T tiles + AV accumulation ----
                  pav = pAV.tile([128, H * 33], F32, tag="pAV", name="pav")
                  Tslabs = []
                  for J in range(I + 1):
                      pTt = pT.tile([128, 1024], BF16, tag="pT", name="pTt")
                      for h in range(H):
                          nc.tensor.transpose(
                              pTt[:, h * 128:(h + 1) * 128],
                              bA[:, h * w + J * 128: h * w + J * 128 + 128],
                              ident[:])
                      Ts = sm.tile([128, 1024], BF16, tag="Tslab",
                                   name="Ts", bufs=4)
                      if not strict:
                          nc.scalar.activation(Ts[:], pTt[:], Act.Exp)
                      else:
                          # z^T = c_j * t^T: c is per-partition post-transpose
                          # but differs per head -> per-head DVE scalar mults
                          # into Ts, then ONE batched in-place exp.
                          csb = csbs[b * 4 + J]
                          for h in range(H):
                              nc.vector.tensor_scalar(
                                  Ts[:, h * 128:(h + 1) * 128],
                                  pTt[:, h * 128:(h + 1) * 128],
                                  csb[:, h:h + 1], None, op0=Alu.mult)
                          nc.scalar.activation(Ts[:], Ts[:], Act.Exp)
                      if strict and I == 0:
                          # zero_pad: P[i=0, :] = 0  (col i=0 of P^T tiles)
                          Tv = Ts.rearrange("p (h c) -> p h c", c=128)
                          nc.gpsimd.memset(Tv[:, :, 0:1], 0.0)
                      Tslabs.append(Ts)
                  orow = ones0 if I == 0 else ones1
                  for h in range(H):
                      hs = slice(h * 33, (h + 1) * 33)
                      for J in range(I + 1):
                          nc.tensor.matmul(
                              pav[:, hs], Tslabs[J][:, h * 128:(h + 1) * 128],
                              V[b * 4 + J][:, hs],
                              start=(J == 0), stop=(J == 3))
                      for J in range(I + 1, 4):
                          if not strict:
                              nc.tensor.matmul(
                                  pav[:, hs],
                                  Ep[h][J][b][:, I * 128:(I + 1) * 128],
                                  V[b * 4 + J][:, hs],
                                  start=False, stop=(J == 3))
                          else:
                              nc.tensor.matmul(
                                  pav[:, hs], orow[0:1, 0:128],
                                  wsb[b][J][0:1, hs],
                                  start=False, stop=(J == 3))
                  # ---- softmax-2 normalize + Wo + resid + LN stats ----
                  pav3 = pav.rearrange("p (h c) -> p h c", c=33)
                  rsm = sm.tile([128, H], F32, tag="rsm", name="rsm")
                  nc.vector.tensor_scalar(rsm[:], pav3[:, :, 32:33],
                                          1e-30, None, op0=Alu.max)
                  rsi = sm.tile([128, H], F32, tag="rsi", name="rsi")
                  nc.vector.reciprocal(rsi[:], rsm[:])
                  att_sb = sm.tile([128, 256], BF16, tag="att",
                                   name="att", bufs=2)
                  nc.vector.tensor_copy(
                      att_sb.rearrange("p (h c) -> p h c", c=32),
                      pav3[:, :, 0:32])
                  att_n = sm.tile([128, 256], BF16, tag="attn",
                                  name="attn", bufs=2)
                  for h in range(H):
                      nc.vector.tensor_scalar(
                          att_n[:, h * 32:(h + 1) * 32],
                          att_sb[:, h * 32:(h + 1) * 32],
                          rsi[:, h:h + 1], None, op0=Alu.mult)
                  attT = [sm.tile([128, 128], BF16, tag=f"attT{i}",
                                  name="attT", bufs=2) for i in range(2)]
                  for dc in range(2):
                      transpose128(attT[dc][:],
                                   att_n[:, dc * 128:(dc + 1) * 128])
                  po = pO.tile([128, D], F32, tag="pO", name="po")
                  nc.tensor.matmul(po[:], attT[0][:], Wo[0][:],
                                   start=True, stop=False)
                  nc.tensor.matmul(po[:], attT[1][:], Wo[1][:],
                                   start=False, stop=False)
                  nc.tensor.matmul(po[:], ones1[0:1, 0:128], obr[:],
                                   start=False, stop=True)
                  y = sm.tile([128, D], BF16, tag="ybuf", name="y", bufs=8)
                  if resid_dram is not None:
                      resid = sm.tile([128, D], F32, tag="xin",
                                      name="resid", bufs=3)
                      nc.sync.dma_start(
                          resid[:], resid_dram[b, I * 128:(I + 1) * 128, :])
                      nc.vector.tensor_tensor(y[:], po[:], resid[:],
                                              op=Alu.add)
                  else:
                      nc.vector.tensor_copy(y[:], po[:])
                  ybufs.append(y)
                  st6 = sm.tile([128, 6], F32, tag="st6", name="st6")
                  nc.vector.bn_stats(st6[:], y[:])
                  nc.vector.bn_aggr(mvh[:, 2 * I:2 * I + 2], st6[:])
              # ---- LN apply for this half (gamma/beta folded) ----
              lnv = sm.tile([128, 4], F32, tag="std", name="lnv", bufs=2)
              nc.scalar.activation(lnv[:], mvh[:, 1:8:2], Act.Ln,
                                   bias=epsT[:], scale=1.0)
              rstd = sm.tile([128, 4], F32, tag="rstd", name="rstd", bufs=2)
              nc.scalar.activation(rstd[:], lnv[:], Act.Exp, scale=-0.5)
              for I in range(4):
                  hout = work.tile([128, D], BF16, tag="hblk", bufs=8,
                                   name=f"{out_name}_{b}_{I}")
                  nc.vector.tensor_scalar(
                      hout[:], ybufs[I][:], mvh[:, 2 * I:2 * I + 1],
                      rstd[:, I:I + 1], op0=Alu.subtract, op1=Alu.mult)
                  houts.append(hout)
              return houts

          def transpose_chunks(chunks, out_name):
              out = [work.tile([128, BL * S], BF16, tag="xfrm", bufs=6,
                               name=f"{out_name}_{dc}") for dc in range(2)]
              for pc in range(PC):
                  for dc in range(2):
                      transpose128(out[dc][:, pc * 128:(pc + 1) * 128],
                                   chunks[pc][:, dc * 128:(dc + 1) * 128])
              return out

          # ================= blocks 1, 2 (interleaved) =================
          xT1 = [xT[(1, 0)], xT[(1, 1)]]
          QT1 = projQ(xT1, 'b1_wq', 'b1_qbr', 'QT1')
          V1 = projN(xT1, 'b1_wv', 'b1_vbr', 'V1')
          prep_x(2, x2d)
          xT2 = [xT[(2, 0)], xT[(2, 1)]]
          QT2 = projQ(xT2, 'b2_wq', 'b2_qbr', 'QT2')

          # alpha = softmax_h(x1 . keyh): fill-in work for the ramp-up
          alphas = []
          for pc in range(PC):
              psb = pO.tile([128, D], F32, tag="pO", name="psb")
              for kc in range(2):
                  nc.tensor.matmul(psb[:, 0:H],
                                   xT[(1, kc)][:, pc * 128:(pc + 1) * 128],
                                   C[f"keyhT__{kc}"][:],
                                   start=(kc == 0), stop=(kc == 1))
              ea = sm.tile([128, H], F32, tag="ea", name="ea")
              rsa = sm.tile([128, 1], F32, tag="rsa", name="rsa")
              nc.scalar.activation(ea[:], psb[:, 0:H], Act.Exp,
                                   accum_out=rsa[:])
              ira = sm.tile([128, 1], F32, tag="ira", name="ira")
              nc.vector.reciprocal(ira[:], rsa[:])
              al = sm.tile([128, H], F32, tag="alpha", name="alpha", bufs=8)
              nc.vector.tensor_scalar(al[:], ea[:], ira[:], None,
                                      op0=Alu.mult)
              alphas.append(al)

          st1 = phase1('b1', QT1, 0)
          st1 = phase1('b1', QT1, 1, st=st1)
          tok1 = mk_token([st1['Ep'][H - 1][3][1]])
          phase2_sqrt(st1, 0, token=tok1)
          phase2_sqrt(st1, 1, token=tok1)
          phase2_exp(st1, 0)
          phase2_exp(st1, 1)
          hq0 = phase3('b1', st1, QT1, V1, 0, resid_dram=x1d, out_name='hq')
          st2 = phase1('b2', QT2, 0)
          hq1 = phase3('b1', st1, QT1, V1, 1, resid_dram=x1d, out_name='hq')
          st2 = phase1('b2', QT2, 1, st=st2)
          hq = hq0 + hq1
          hqT = transpose_chunks(hq, 'hqT')
          V2 = projN(xT2, 'b2_wv', 'b2_vbr', 'V2')
          tok2 = mk_token([st2['Ep'][H - 1][3][1], hq1[3]])
          phase2_sqrt(st2, 0, token=tok2)
          phase2_sqrt(st2, 1, token=tok2)
          phase2_exp(st2, 0)
          phase2_exp(st2, 1)
          ha0 = phase3('b2', st2, QT2, V2, 0, resid_dram=x2d, out_name='ha')
          ha1 = phase3('b2', st2, QT2, V2, 1, resid_dram=x2d, out_name='ha')
          ha = ha0 + ha1
          haT = transpose_chunks(ha, 'haT')

          # ================= block 3 =================
          K3T = projT(hqT, 'b3_wk', 'b3_kbT', 'K3T')
          V3 = projN(haT, 'b3_wv', 'b3_vbr', 'V3')
          # c rows -> cTt [1, 4*1024], col (h%4)*1024 + b*512 + i (per grp)
          csbs, csbbs, ecs = [], [], []  # c (f32), c (bf16), exp(c)
          ctparts = {}
          for pc in range(PC):
              psc = p1.tile([128, 1024], F32, tag="p1", name="psc")[:, 0:H]
              for kc in range(2):
                  nc.tensor.matmul(psc[:],
                                   K3T[kc][:, pc * 128:(pc + 1) * 128],
                                   C[f"q03__{kc}"][:],
                                   start=(kc == 0), stop=(kc == 1))
              csb = sm.tile([128, H], F32, tag="csb", name="csb", bufs=8)
              nc.vector.tensor_copy(csb[:], psc[:])
              csbs.append(csb)
              csbb = sm.tile([128, H], BF16, tag="csbb", name="csbb", bufs=8)
              nc.vector.tensor_copy(csbb[:], psc[:])
              csbbs.append(csbb)
              ec = sm.tile([128, H], BF16, tag="ec", name="ec", bufs=8)
              nc.scalar.activation(ec[:], psc[:], Act.Exp)
              ecs.append(ec)
          # Eb3[h][b] [128, 512] = exp(c) broadcast along partitions (the
          # scan input): c^T via PE transpose, ones-broadcast matmul, exp.
          # Stored as halves of the Ep{hp}_3-shaped pair tiles.
          Eb3 = [[None] * BL for _ in range(H)]
          for hp in range(H // 2):
              pair = [work.tile([128, 1024], BF16, tag=f"Ep{hp}_3",
                                bufs=2, name=f"Eb3_{hp}_{b}")
                      for b in range(BL)]
              for sub in range(2):
                  h = 2 * hp + sub
                  ptc = pT.tile([128, 1024], BF16, tag="pT", name="ptc")
                  for pc in range(PC):
                      nc.tensor.transpose(ptc[0:1, pc * 128:(pc + 1) * 128],
                                          csbbs[pc][:, h:h + 1], ident[:])
                  cTh = rot.tile([1, 1024], BF16, tag="cT", bufs=2,
                                 name=f"cT_{h}")
                  nc.vector.tensor_copy(cTh[:], ptc[0:1, 0:1024])
                  ps = p1.tile([128, 1024], F32, tag="p1", name="bc_ps")
                  for b in range(BL):
                      nc.tensor.matmul(
                          ps[:, b * 512:(b + 1) * 512], ones1[0:1, 0:128],
                          cTh[0:1, b * 512:(b + 1) * 512],
                          start=True, stop=True)
                  for b in range(BL):
                      nc.scalar.activation(
                          pair[b][:, sub * 512:(sub + 1) * 512],
                          ps[:, b * 512:(b + 1) * 512], Act.Exp)
                      Eb3[h][b] = pair[b][:, sub * 512:(sub + 1) * 512]
          # rank-1 upper contributions: w[b][J][1, 264] = sum_j exp(c_j) V[j]
          wsb = [[None] * 4 for _ in range(BL)]
          for b in range(BL):
              for J in range(1, 4):
                  pc = b * 4 + J
                  pw = p1.tile([128, 1024], F32, tag="p1",
                               name="pw")[0:1, 0:H * 33]
                  for h in range(H):
                      nc.tensor.matmul(pw[0:1, h * 33:(h + 1) * 33],
                                       ecs[pc][:, h:h + 1],
                                       V3[pc][:, h * 33:(h + 1) * 33],
                                       start=True, stop=True)
                  wt = sm.tile([1, H * 33], BF16, tag="wsb", name="wsb",
                               bufs=6)
                  nc.vector.tensor_copy(wt[:], pw[:])
                  wsb[b][J] = wt

          st3 = phase1('b3', None, 0, Eb3=Eb3)
          st3 = phase1('b3', None, 1, st=st3, Eb3=Eb3)
          tok3 = mk_token([Eb3[H - 1][1], ha1[3]])
          phase2_sqrt(st3, 0, token=tok3)
          phase2_sqrt(st3, 1, token=tok3)
          phase2_exp(st3, 0)
          phase2_exp(st3, 1)
          h30 = phase3('b3', st3, None, V3, 0, csbs=csbs, wsb=wsb,
                       out_name='h3')
          h31 = phase3('b3', st3, None, V3, 1, csbs=csbs, wsb=wsb,
                       out_name='h3')
          h3 = h30 + h31

          # ================= final stage =================
          # per-head transposed h3: h3Tp[grp] [32, 4*1024],
          # col (h%4)*1024 + pc*128
          h3Tp = [rot.tile([32, 4 * 1024], BF16, tag="h3Tp", bufs=2,
                           name=f"h3Tp_{grp}") for grp in range(2)]
          for pc in range(PC):
              for grp in range(2):
                  ptv = pT.tile([128, 1024], BF16, tag="pT", name="ptv")
                  for hh in range(4):
                      h = grp * 4 + hh
                      nc.tensor.transpose(ptv[0:32, hh * 128:(hh + 1) * 128],
                                          h3[pc][:, h * 32:(h + 1) * 32],
                                          ident[:])
                  src3 = ptv[0:32, 0:512].rearrange("p (h c) -> p h c", h=4)
                  dview = h3Tp[grp].rearrange("p (h c) -> p h c", h=4)[
                      0:32, :, pc * 128:pc * 128 + 128]
                  nc.vector.tensor_copy(dview, src3)
          for pc in range(PC):
              b, ic = divmod(pc, 4)
              vhalves = []
              for half in range(2):
                  ps = p1.tile([128, 1024], F32, tag="p1", name="val_ps")
                  for hh in range(4):
                      h = half * 4 + hh
                      seg = ps[:, hh * 256:(hh + 1) * 256]
                      nc.tensor.matmul(
                          seg,
                          h3Tp[half][0:32, hh * 1024 + pc * 128:
                                     hh * 1024 + pc * 128 + 128],
                          C[f"lvw__{h}"][:],
                          start=True, stop=False)
                      nc.tensor.matmul(seg, ones1[0:1, 0:128],
                                       C['lvbr'][0:1, h * 256:(h + 1) * 256],
                                       start=False, stop=True)
                  val = rot.tile([128, 1024], BF16, tag="val",
                                 name="val", bufs=2)
                  nc.scalar.activation(val[:], ps[:], Act.Sigmoid)
                  vhalves.append(val)
              alpha = alphas[pc]
              acc = sm.tile([128, D], F32, tag="acc", name="acc", bufs=2)
              nc.vector.tensor_scalar(acc[:], vhalves[0][:, 0:256],
                                      alpha[:, 0:1], None, op0=Alu.mult)
              for h in range(1, H):
                  half, hh = divmod(h, 4)
                  acc2 = sm.tile([128, D], F32, tag="acc", name="acc2",
                                 bufs=2)
                  nc.vector.scalar_tensor_tensor(
                      acc2[:], vhalves[half][:, hh * 256:(hh + 1) * 256],
                      alpha[:, h:h + 1], acc[:],
                      op0=Alu.mult, op1=Alu.add)
                  acc = acc2
              nc.sync.dma_start(outd[b, ic * 128:(ic + 1) * 128, :], acc[:])

    nc.compile()
    return nc


_GRAPH_CACHE = {}


def _get_graph(consts, g2):
    key = tuple(np.float32(v) for blk in ('b1', 'b2', 'b3')
                for v in g2[blk])
    if key not in _GRAPH_CACHE:
        _GRAPH_CACHE[key] = _build(consts, g2)
    return _GRAPH_CACHE[key]


def kernel(**inputs):
    consts, g2 = _host_prep(inputs)
    nc = _get_graph(consts, g2)
    q = np.ascontiguousarray(np.asarray(inputs['q_emb'], np.float32))
    qa = np.ascontiguousarray(np.asarray(inputs['qa_emb'], np.float32))
    in_maps = []
    for core in range(NCORES):
        m = {'x1': q[core * BL:(core + 1) * BL],
             'x2': qa[core * BL:(core + 1) * BL]}
        m.update(consts)
        in_maps.append(m)
    res = run_bass_kernel_spmd(nc, in_maps, core_ids=list(range(NCORES)))
    out = np.concatenate([res.results[c]['out'] for c in range(NCORES)],
                         axis=0)
    return out.astype(np.float32)


# revision 54
# speedup vs baseline: 6.1097x; 6.1097x over previous
"""Trainium2 Bass kernel for nn_Architecture_11879879540882 (AKT-style
monotonic sparse attention), data-parallel over batch on 8 NeuronCores.

Self-contained: hardcodes shapes B=16,S=512,D=256,H=8,DK=32, shards the batch
2-per-core, runs one Bass graph SPMD via run_bass_kernel_spmd, gathers output.

Algorithm notes (validated vs the jax reference):
 - blocks 1/2: k-projection shares q weights and inputs -> K == Q, so the
   score matrix is SYMMETRIC.
 - masked softmax + cumsum distance statistics collapse into ONE reversed
   masked scan: state=(E+state)*mask -> all suffix sums + masked row total.
 - dist = sqrt(suffix*pos*g^2/r), total = exp(-dist)   [g = -softplus(gamma)]
 - second softmax is UNMASKED (reproduces the reference's non-inplace
   masked_fill bug); the 1e-5 clip on `total` is skipped (~1e-4 error).
 - all LayerNorm gamma/beta fold into downstream weights on host.
 - block3's query is position-independent -> its score rows are broadcasts.

Redesign vs the first working version (2.x speedup targets):
 - TRIANGLE NARROWING: for row-chunk I (128 rows), cols >= (I+1)*128 have
   total==1, so phase-1 exp/scan/stat-write and phase-2 sqrt/exp only cover
   (I+1)*128 cols (62.5% of full).
 - TRANSPOSE-BEFORE-EXP: phase 3 transposes z=s*total (PE), then the ACT
   exp reads the transposed PSUM directly -> P^T lands in SBUF with no
   separate PSUM->SBUF copy instruction.
 - SYMMETRY REUSE: for b1/b2 the strict-upper P^T tiles equal phase-1's
   exp(s) tiles (kept in SBUF); for b3 they are rank-1 (broadcast scores)
   and collapse into tiny [1,x] matmuls + a ones-broadcast matmul.
 - ENGINE REBALANCE: distance-stat writes + att copies + final accumulate
   move to the (otherwise idle) Pool engine; bigA is reused in place for
   sqrt/exp/z (saves ~36KB/partition of SBUF).
"""
import sys
import numpy as np

for _p in ('/opt/trn_rl_repo',):
    if _p not in sys.path:
        sys.path.append(_p)

import ml_dtypes
import concourse.bass as bass
import concourse.bacc as bacc
import concourse.tile as tile
import concourse.mybir as mybir
from concourse.bass_utils import run_bass_kernel_spmd

F32 = mybir.dt.float32
BF16 = mybir.dt.bfloat16
Alu = mybir.AluOpType
Act = mybir.ActivationFunctionType
NPBF = ml_dtypes.bfloat16

B, S, D, H, DK = 16, 512, 256, 8, 32
NCORES = 8
BL = B // NCORES          # local batches per core = 2
PC = BL * 4               # 128-row position chunks per core = 8
LN_EPS = 1e-5

REV = (slice(None), slice(None, None, -1))

# packed-constant layouts: (name, ncols); all [128, ncols] bf16.
PACKA = ([('ident', 128)]
         + [(f'b1_{w}__{k}', 256) for w in ('wq', 'wv', 'wo')
            for k in range(2)]
         + [(f'mpad__{i}', 513) for i in range(4)]
         + [(f'posm__{i}', 512) for i in range(4)])
PACKB = ([(f'b2_{w}__{k}', 256) for w in ('wq', 'wv', 'wo')
          for k in range(2)]
         + [(f'b3_{w}__{k}', 256) for w in ('wk', 'wv', 'wo')
            for k in range(2)]
         + [(f'q03__{k}', 8) for k in range(2)]
         + [(f'keyhT__{k}', 8) for k in range(2)])
PACK32 = [('b3_kbT', 2), ('g2b_b1', 8), ('g2b_b2', 8), ('g2b_b3', 8)]


def _softplus(x):
    return np.logaddexp(0.0, x)


def _host_prep(inp):
    """Parameter preprocessing on host. Returns (consts dict, g2 dict)."""
    p = {k: np.asarray(v, np.float32) for k, v in inp.items()}
    c = {}
    s4 = np.float32(DK ** -0.25)
    bf = lambda x: np.ascontiguousarray(np.asarray(x, np.float32)).astype(NPBF)
    colpack = lambda b: np.ascontiguousarray(
        np.asarray(b, np.float32).reshape(2, 128).T).astype(np.float32)

    for blk in ('b1', 'b2'):
        c[blk + '_wq'] = bf(p[blk + '_qw'] * s4)
        c[blk + '_qbr'] = bf((p[blk + '_qb'] * s4)[None, :])
        c[blk + '_wv'] = bf(p[blk + '_vw'])
        c[blk + '_vbr'] = bf(p[blk + '_vb'][None, :])
        c[blk + '_wo'] = bf(p[blk + '_ow'])
        c[blk + '_obr'] = bf(p[blk + '_ob'][None, :])
    know = p['know'][0, 0]
    q03 = ((know @ p['b3_qw'] + p['b3_qb']) / np.sqrt(DK)).reshape(H, DK)
    Q03 = np.zeros((D, H), np.float32)
    for h in range(H):
        Q03[h * DK:(h + 1) * DK, h] = q03[h]
    c['q03'] = bf(Q03)
    g1, be1 = p['b1_lng'], p['b1_lnb']
    c['b3_wk'] = bf(p['b3_kw'] * g1[:, None])
    c['b3_kbT'] = colpack(p['b3_kb'] + be1 @ p['b3_kw'])
    g2_, be2 = p['b2_lng'], p['b2_lnb']
    c['b3_wv'] = bf(p['b3_vw'] * g2_[:, None])
    c['b3_vbr'] = bf((p['b3_vb'] + be2 @ p['b3_vw'])[None, :])
    c['b3_wo'] = bf(p['b3_ow'])
    c['b3_obr'] = bf((p['b3_ob'] + know)[None, :])
    g3, be3 = p['b3_lng'], p['b3_lnb']
    lvw = np.zeros((H, DK, D), np.float32)
    lvb = np.zeros((H, D), np.float32)
    for h in range(H):
        sl = slice(h * DK, (h + 1) * DK)
        lvw[h] = p['lv_w'] * g3[sl][:, None]
        lvb[h] = p['lv_b'] + be3[sl] @ p['lv_w']
    c['lvw'] = bf(lvw)                            # -> lvw__h [32,256]
    c['lvbr'] = bf(lvb.reshape(1, H * D))         # [1, 2048]
    know_r = know.reshape(H, DK)
    keyh = 1.0 / (1.0 + np.exp(-(know_r @ p['lk_w'] + p['lk_b'])))
    c['keyhT'] = bf(keyh.T)                       # [D, H]

    # padded inclusive mask: mpad[ic][p, j] = (j <= i_p), j in [0, 512];
    # strict mask is the shifted view mpad[:, 1:513].
    i = np.arange(S + 1, dtype=np.int64)
    mpad = np.zeros((4, 128, S + 1), np.float32)
    pos = np.zeros((4, 128, S), np.float32)
    for ic in range(4):
        ii = np.arange(ic * 128, (ic + 1) * 128, dtype=np.int64)[:, None]
        mpad[ic] = (i[None, :] <= ii)
        pos[ic] = np.abs(ii - i[None, :S])
    for blk in ('b1', 'b2', 'b3'):
        g2v = (_softplus(p[blk + '_gam'][:, 0, 0]) ** 2).astype(np.float32)
        c['g2b_' + blk] = np.ascontiguousarray(
            np.broadcast_to(g2v[None, :], (128, H))).astype(np.float32)
    c['mpad'] = bf(mpad)
    c['posm'] = bf(pos)
    c['ident'] = bf(np.eye(128))

    flat = {}
    for name, a in c.items():
        if a.ndim == 2 and a.shape[0] > 128:
            for kc in range(a.shape[0] // 128):
                flat[f"{name}__{kc}"] = np.ascontiguousarray(
                    a[kc * 128:(kc + 1) * 128])
        elif a.ndim == 3:
            for kc in range(a.shape[0]):
                flat[f"{name}__{kc}"] = np.ascontiguousarray(a[kc])
        else:
            flat[name] = a
    # pack the [128, x] bf16 consts into two big arrays (2 DMAs instead of
    # ~30 -- the serial SP DMA-issue time dominated kernel startup).
    packed = {}
    for pname, layout in (('packA', PACKA), ('packB', PACKB)):
        tot = sum(w for _, w in layout)
        arr = np.zeros((128, tot), NPBF)
        off = 0
        for nm, wd in layout:
            arr[:, off:off + wd] = flat.pop(nm)
            off += wd
        packed[pname] = arr
    tot = sum(w for _, w in PACK32)
    arr = np.zeros((128, tot), np.float32)
    off = 0
    for nm, wd in PACK32:
        arr[:, off:off + wd] = flat.pop(nm)
        off += wd
    packed['pack32'] = arr
    flat.update(packed)
    g2 = {blk: [float(v) for v in
                (_softplus(p[blk + '_gam'][:, 0, 0]) ** 2)]
          for blk in ('b1', 'b2', 'b3')}
    return flat, g2


_NPDT = {np.dtype(np.float32): F32, np.dtype(NPBF): BF16}


def _build(consts, g2, reps=1):
    """Builds the per-core Bass graph (BL local batches). reps>1 repeats
    the whole computation on-device (for slope-based timing)."""
    nc = bacc.Bacc("TRN2", target_bir_lowering=False, debug=False)

    x1d = nc.dram_tensor("x1", [BL, S, D], F32, kind="ExternalInput")
    x2d = nc.dram_tensor("x2", [BL, S, D], F32, kind="ExternalInput")
    outd = nc.dram_tensor("out", [BL, S, D], F32, kind="ExternalOutput")
    cd = {name: nc.dram_tensor(name, list(a.shape), _NPDT[a.dtype],
                               kind="ExternalInput")
          for name, a in consts.items()}

    from contextlib import ExitStack
    with tile.TileContext(nc) as tc, ExitStack() as _ps:
        sb = _ps.enter_context(tc.tile_pool(name="const", bufs=1))
        work = _ps.enter_context(tc.tile_pool(name="work", bufs=1))
        sm = _ps.enter_context(tc.tile_pool(name="sm", bufs=4))
        rot = _ps.enter_context(tc.tile_pool(name="rot", bufs=4))
        p1 = _ps.enter_context(tc.tile_pool(name="p1", bufs=2, space="PSUM"))
        pT = _ps.enter_context(tc.tile_pool(name="pT", bufs=2, space="PSUM"))
        pAV = _ps.enter_context(tc.tile_pool(name="pAV", bufs=1, space="PSUM"))
        pO = _ps.enter_context(tc.tile_pool(name="pO", bufs=1, space="PSUM"))

        # ---------- constants (packed: 3 big DMAs + a few stragglers) ----
        C = {}
        for pname, layout, dt in (('packA', PACKA, BF16),
                                  ('packB', PACKB, BF16),
                                  ('pack32', PACK32, F32)):
            tot = sum(wd for _, wd in layout)
            t = sb.tile([128, tot], dt, name="c_" + pname)
            nc.sync.dma_start(t[:], cd[pname][:])
            off = 0
            for nm, wd in layout:
                C[nm] = t[:, off:off + wd]
                off += wd

        def _prio(name):
            for i, k in enumerate(('b1_', 'b2_', 'b3_', 'lvbr', 'lv', 'key')):
                if name.startswith(k):
                    return i
            return 99
        for name in sorted(cd, key=_prio):
            if name in ('packA', 'packB', 'pack32'):
                continue
            ap = cd[name]
            t = sb.tile(list(ap.shape), ap.dtype, name="c_" + name)
            nc.sync.dma_start(t[:], ap[:])
            C[name] = t
        ones1 = sb.tile([1, 512], BF16, name="ones1")
        nc.vector.memset(ones1[:], 1.0)
        ones0 = sb.tile([1, 128], BF16, name="ones0")   # [0,1,1,...]
        nc.vector.memset(ones0[:], 1.0)
        nc.vector.memset(ones0[0:1, 0:1], 0.0)
        epsT = sb.tile([128, 1], F32, name="epsT")
        nc.vector.memset(epsT[:], LN_EPS)
        ident = C['ident']
        mpad = [C[f"mpad__{ic}"] for ic in range(4)]
        posm = [C[f"posm__{ic}"] for ic in range(4)]

        def transpose128(dst, src, cp_engine=None):
            """dst[128,128] SBUF bf16 = src.T via PE + copy."""
            pt = pT.tile([128, 1024], BF16, tag="pT", name="pt_t")
            nc.tensor.transpose(pt[:, 0:128], src, ident[:])
            (cp_engine or nc.vector).tensor_copy(dst, pt[:, 0:128])

        # ---------- input prep: transposed bf16 copies of x1/x2 ----------
        # x DMAs issue from the (idle) Pool queue so they run in parallel
        # with the SP queue's long constants sequence.
        for _rep in range(reps):
          xT = {}

          def prep_x(xi, xd):
              for dc in range(2):
                  xT[(xi, dc)] = work.tile([128, BL * S], BF16, tag="xfrm",
                                           bufs=6, name=f"xT{xi}_{dc}")
              for pc in range(PC):
                  b, ic = divmod(pc, 4)
                  t = sm.tile([128, D], F32, tag="xin", name="xin", bufs=3)
                  nc.gpsimd.dma_start(t[:], xd[b, ic * 128:(ic + 1) * 128, :])
                  tb = sm.tile([128, D], BF16, tag="xbf", name="xbf_t", bufs=2)
                  nc.vector.tensor_copy(tb[:], t[:])
                  for dc in range(2):
                      transpose128(xT[(xi, dc)][:, pc * 128:(pc + 1) * 128],
                                   tb[:, dc * 128:(dc + 1) * 128])

          prep_x(1, x1d)

          # ---------- projections ----------
          def projQ(xTloc, wname, brname, out_name):
              """Head-packed transposed projection QTp [64, H/2*BL*S]:
              head h sits at partition base 32*(h%2),
              cols (h//2)*1024 + b*512 + pos."""
              QTp = work.tile([64, (H // 2) * BL * S], BF16, name=out_name,
                              tag="QTp", bufs=2)
              qbr = C[brname]
              for hp in range(H // 2):
                  for half in range(BL):
                      ps = p1.tile([128, 1024], F32, tag="p1",
                                   name="projQ_ps")
                      for sub in range(2):
                          h = 2 * hp + sub
                          bp = 32 * sub
                          for kc in range(2):
                              nc.tensor.matmul(
                                  ps[bp:bp + 32, 0:512],
                                  C[f"{wname}__{kc}"][:, h * 32:(h + 1) * 32],
                                  xTloc[kc][:, half * 512:(half + 1) * 512],
                                  start=(kc == 0), stop=False)
                          nc.tensor.matmul(ps[bp:bp + 32, 0:512],
                                           qbr[0:1, h * 32:(h + 1) * 32],
                                           ones1[0:1, :], start=False,
                                           stop=True)
                      nc.vector.tensor_copy(
                          QTp[0:64, hp * 1024 + half * 512:
                              hp * 1024 + (half + 1) * 512],
                          ps[0:64, 0:512])
              return QTp

          def projT(xTloc, wname, bTname, out_name):
              """Chunk-transposed projection out[dc][128, BL*S] (for K3T)."""
              out = [work.tile([128, BL * S], BF16, tag="xfrm", bufs=6,
                               name=f"{out_name}_{dc}") for dc in range(2)]
              bT = C[bTname]
              for dc in range(2):
                  for hh in range(BL):
                      ps = p1.tile([128, 1024], F32, tag="p1", name="projT_ps")
                      for kc in range(2):
                          nc.tensor.matmul(
                              ps[:, 0:512],
                              C[f"{wname}__{kc}"][:, dc * 128:(dc + 1) * 128],
                              xTloc[kc][:, hh * 512:(hh + 1) * 512],
                              start=(kc == 0), stop=(kc == 1))
                      nc.scalar.activation(out[dc][:, hh * 512:(hh + 1) * 512],
                                           ps[:, 0:512], Act.Identity,
                                           bias=bT[:, dc:dc + 1], scale=1.0)
              return out

          def projN(xTloc, wname, brname, out_name):
              """Natural projection, head-packed with a ones column:
              out[pc] [128, 8*33]: head h = cols [33h, 33h+32), col 33h+32=1.
              The ones column makes the AV matmul also emit softmax-2 row
              sums."""
              out = [work.tile([128, H * 33], BF16, tag="Vt", bufs=12,
                               name=f"{out_name}_{pc}") for pc in range(PC)]
              br = C[brname]
              for pc in range(PC):
                  ps = pO.tile([128, D], F32, tag="pO", name="projN_ps")
                  for kc in range(2):
                      nc.tensor.matmul(ps[:],
                                       xTloc[kc][:, pc * 128:(pc + 1) * 128],
                                       C[f"{wname}__{kc}"],
                                       start=(kc == 0), stop=False)
                  nc.tensor.matmul(ps[:], ones1[0:1, 0:128], br[:],
                                   start=False, stop=True)
                  ov = out[pc].rearrange("p (h c) -> p h c", c=33)
                  nc.vector.tensor_copy(ov[:, :, 0:32],
                                        ps.rearrange("p (h c) -> p h c",
                                                     c=32))
                  nc.gpsimd.memset(ov[:, :, 32:33], 1.0)
              return out

          # ============== attention block phases ==============
          # state[(b)] = dict with Ep tiles, bigA tiles, etc.

          def qk_ph1(QTp, h, I, b, ps, w):
              """phase-1 scores ps[:, 0:w] for one batch-half."""
              hp, sub = divmod(h, 2)
              bp = 32 * sub
              base = hp * 1024 + b * 512
              nc.tensor.matmul(
                  ps[:, 0:w],
                  QTp[bp:bp + 32, base + I * 128: base + I * 128 + 128],
                  QTp[bp:bp + 32, base: base + w],
                  start=True, stop=True)

          def phase1(blk, QTp, b, st=None, Eb3=None):
              """QK + exp + scans + stat-writes for ONE batch-half b.
              Accumulates into/returns state st: Ep[h][I][b], bigA[b][I]."""
              strict = (blk == 'b3')
              g2b = C['g2b_' + blk]
              if st is None:
                  st = {'Ep': [[[None] * BL for _ in range(4)]
                               for _ in range(H)],
                        'bigA': [[None] * 4 for _ in range(BL)],
                        'strict': strict}
              Ep, bigA = st['Ep'], st['bigA']
              for I in range(4):
                  w = (I + 1) * 128
                  bigA[b][I] = work.tile([128, H * w], BF16,
                                         tag=f"bigA{I}", bufs=2,
                                         name=f"bigA_{blk}_{b}_{I}")
              if strict:
                  # Eb3[h][b] = exp(broadcast score row), reused for all I.
                  for h in range(H):
                      for I in range(4):
                          Ep[h][I][b] = Eb3[h][b]
              for I in range(4):
                  w = (I + 1) * 128
                  maskv = (mpad[I][:, 1:w + 1] if strict
                           else mpad[I][:, 0:w])
                  r8 = sm.tile([128, H], F32, tag="r8", name="r8", bufs=2)
                  Rvs = {}
                  if not strict:
                      for hp in range(H // 2):
                          ps = p1.tile([128, 1024], F32, tag="p1",
                                       name="qk1_ps")
                          for sub in range(2):
                              h = 2 * hp + sub
                              bp = 32 * sub
                              base = hp * 1024 + b * 512
                              nc.tensor.matmul(
                                  ps[:, sub * 512: sub * 512 + w],
                                  QTp[bp:bp + 32,
                                      base + I * 128: base + I * 128 + 128],
                                  QTp[bp:bp + 32, base: base + w],
                                  start=True, stop=True)
                          e = work.tile([128, 2 * w], BF16,
                                        tag=f"Ep{hp}_{I}", bufs=2,
                                        name=f"Ep_{hp}_{I}_{b}")
                          nc.scalar.activation(
                              e.rearrange("p (s c) -> p s c", c=w),
                              ps.rearrange("p (s c) -> p s c",
                                           c=512)[:, :, 0:w], Act.Exp)
                          Ep[2 * hp][I][b] = e[:, 0:w]
                          Ep[2 * hp + 1][I][b] = e[:, w:2 * w]
                  for h in range(H):
                      Rv = sm.tile([128, 512], BF16, tag="Rv",
                                   name="Rv", bufs=8)
                      nc.vector.tensor_tensor_scan(
                          Rv[:, 0:w][REV], Ep[h][I][b][:, 0:w][REV],
                          maskv[REV], 0.0, op0=Alu.add, op1=Alu.mult)
                      nc.vector.tensor_copy(r8[:, h:h + 1], Rv[:, 0:1])
                      Rvs[h] = Rv
                  r8m = sm.tile([128, H], F32, tag="r8m", name="r8m", bufs=2)
                  nc.vector.tensor_scalar(r8m[:], r8[:], 1e-30, None,
                                          op0=Alu.max)
                  rc = sm.tile([128, H], F32, tag="rc", name="rc", bufs=2)
                  nc.vector.reciprocal(rc[:], r8m[:])
                  rgb = sm.tile([128, H], F32, tag="rgb", name="rgb", bufs=2)
                  nc.vector.tensor_tensor(rgb[:], rc[:], g2b[:], op=Alu.mult)
                  bA = bigA[b][I]
                  for h in range(H):
                      nc.vector.scalar_tensor_tensor(
                          bA[:, h * w: h * w + w - 1],
                          posm[I][:, 0:w - 1], rgb[:, h:h + 1],
                          Rvs[h][:, 1:w],
                          op0=Alu.mult, op1=Alu.mult)
                  # col w-1 of every head segment: suffix beyond row end
                  # is empty -> bigA = 0 (gives total == 1 post-exp).
                  bAv = bA.rearrange("p (h c) -> p h c", c=w)
                  nc.gpsimd.memset(bAv[:, :, w - 1:w], 0.0)
              return st

          def mk_token(srcs):
              """[128,1] zero tile data-dependent on srcs (ACT-order fence:
              used as a sqrt bias so the sqrt batch can't start, and hence
              can't interleave table-thrashing, before srcs complete)."""
              tok = sm.tile([128, 1], F32, tag="tok", name="tok", bufs=2)
              if len(srcs) == 1:
                  nc.vector.tensor_scalar(tok[:], srcs[0][:, 0:1], 0.0, None,
                                          op0=Alu.mult)
              else:
                  nc.vector.scalar_tensor_tensor(
                      tok[:], srcs[0][:, 0:1], 0.0, srcs[1][:, 0:1],
                      op0=Alu.mult, op1=Alu.mult)
              return tok

          def phase2_sqrt(st, b, token=None):
              for I in range(4):
                  bA = st['bigA'][b][I]
                  if token is not None:
                      nc.scalar.activation(bA[:], bA[:], Act.Sqrt,
                                           bias=token[:, 0:1], scale=1.0)
                  else:
                      nc.scalar.activation(bA[:], bA[:], Act.Sqrt)

          def phase2_exp(st, b):
              for I in range(4):
                  bA = st['bigA'][b][I]
                  nc.scalar.activation(bA[:], bA[:], Act.Exp, scale=-1.0)

          def phase3(blk, st, QTp, V, b, csbs=None, wsb=None,
                     resid_dram=None, out_name="hout"):
              """z -> transpose -> exp(P^T) -> AV -> Wo -> +resid -> LN stats
              for one batch-half. Returns 4 hout tiles.

              b1/b2: z = s*t in place on bigA, then transpose+exp.
              b3: scores are broadcast rows c_j, so z^T = c_j * t^T with c_j
              PER-PARTITION after the transpose -> fold into the exp's scale
              operand (no z multiply, no broadcast-score tiles at all)."""
              strict = st['strict']
              Ep = st['Ep']
              Wo = [C[blk + '_wo__0'], C[blk + '_wo__1']]
              obr = C[blk + '_obr']
              houts = []
              mvh = sm.tile([128, 8], F32, tag="mvh", name="mvh", bufs=2)
              ybufs = []
              for I in range(4):
                  w = (I + 1) * 128
                  bA = st['bigA'][b][I]
                  # ---- z = s * total, in place on bigA (b1/b2 only) ----
                  if not strict:
                      for hp in range(4):
                          ps3 = p1.tile([128, 1024], F32, tag="p1",
                                        name="qk3_ps")
                          for sub in range(2):
                              h = 2 * hp + sub
                              hq, s2 = divmod(h, 2)
                              bp = 32 * s2
                              base = hq * 1024 + b * 512
                              nc.tensor.matmul(
                                  ps3[:, sub * 512: sub * 512 + w],
                                  QTp[bp:bp + 32,
                                      base + I * 128: base + I * 128 + 128],
                                  QTp[bp:bp + 32, base: base + w],
                                  start=True, stop=True)
                          zv = bA[:, 2 * hp * w: (2 * hp + 2) * w].rearrange(
                              "p (h c) -> p h c", c=w)
                          psv = ps3.rearrange("p (h c) -> p h c",
                                              c=512)[:, :, 0:w]
                          nc.vector.tensor_tensor(zv, psv, zv, op=Alu.mult)
                  # ---- transposed P^T tiles + AV accumulation ----
                  pav = pAV.tile([128, H * 33], F32, tag="pAV", name="pav")
                  Tslabs = []
                  for J in range(I + 1):
                      pTt = pT.tile([128, 1024], BF16, tag="pT", name="pTt")
                      for h in range(H):
                          nc.tensor.transpose(
                              pTt[:, h * 128:(h + 1) * 128],
                              bA[:, h * w + J * 128: h * w + J * 128 + 128],
                              ident[:])
                      Ts = sm.tile([128, 1024], BF16, tag="Tslab",
                                   name="Ts", bufs=4)
                      if not strict:
                          nc.scalar.activation(Ts[:], pTt[:], Act.Exp)
                      else:
                          # z^T = c_j * t^T: c is per-partition post-transpose
                          # but differs per head -> per-head DVE scalar mults
                          # into Ts, then ONE batched in-place exp.
                          csb = csbs[b * 4 + J]
                          for h in range(H):
                              nc.vector.tensor_scalar(
                                  Ts[:, h * 128:(h + 1) * 128],
                                  pTt[:, h * 128:(h + 1) * 128],
                                  csb[:, h:h + 1], None, op0=Alu.mult)
                          nc.scalar.activation(Ts[:], Ts[:], Act.Exp)
                      if strict and I == 0:
                          # zero_pad: P[i=0, :] = 0  (col i=0 of P^T tiles)
                          Tv = Ts.rearrange("p (h c) -> p h c", c=128)
                          nc.gpsimd.memset(Tv[:, :, 0:1], 0.0)
                      Tslabs.append(Ts)
                  orow = ones0 if I == 0 else ones1
                  for h in range(H):
                      hs = slice(h * 33, (h + 1) * 33)
                      for J in range(I + 1):
                          nc.tensor.matmul(
                              pav[:, hs], Tslabs[J][:, h * 128:(h + 1) * 128],
                              V[b * 4 + J][:, hs],
                              start=(J == 0), stop=(J == 3))
                      for J in range(I + 1, 4):
                          if not strict:
                              nc.tensor.matmul(
                                  pav[:, hs],
                                  Ep[h][J][b][:, I * 128:(I + 1) * 128],
                                  V[b * 4 + J][:, hs],
                                  start=False, stop=(J == 3))
                          else:
                              nc.tensor.matmul(
                                  pav[:, hs], orow[0:1, 0:128],
                                  wsb[b][J][0:1, hs],
                                  start=False, stop=(J == 3))
                  # ---- softmax-2 normalize + Wo + resid + LN stats ----
                  pav3 = pav.rearrange("p (h c) -> p h c", c=33)
                  rsm = sm.tile([128, H], F32, tag="rsm", name="rsm")
                  nc.vector.tensor_scalar(rsm[:], pav3[:, :, 32:33],
                                          1e-30, None, op0=Alu.max)
                  rsi = sm.tile([128, H], F32, tag="rsi", name="rsi")
                  nc.vector.reciprocal(rsi[:], rsm[:])
                  att_sb = sm.tile([128, 256], BF16, tag="att",
                                   name="att", bufs=2)
                  nc.vector.tensor_copy(
                      att_sb.rearrange("p (h c) -> p h c", c=32),
                      pav3[:, :, 0:32])
                  att_n = sm.tile([128, 256], BF16, tag="attn",
                                  name="attn", bufs=2)
                  for h in range(H):
                      nc.vector.tensor_scalar(
                          att_n[:, h * 32:(h + 1) * 32],
                          att_sb[:, h * 32:(h + 1) * 32],
                          rsi[:, h:h + 1], None, op0=Alu.mult)
                  attT = [sm.tile([128, 128], BF16, tag=f"attT{i}",
                                  name="attT", bufs=2) for i in range(2)]
                  for dc in range(2):
                      transpose128(attT[dc][:],
                                   att_n[:, dc * 128:(dc + 1) * 128])
                  po = pO.tile([128, D], F32, tag="pO", name="po")
                  nc.tensor.matmul(po[:], attT[0][:], Wo[0][:],
                                   start=True, stop=False)
                  nc.tensor.matmul(po[:], attT[1][:], Wo[1][:],
                                   start=False, stop=False)
                  nc.tensor.matmul(po[:], ones1[0:1, 0:128], obr[:],
                                   start=False, stop=True)
                  y = sm.tile([128, D], BF16, tag="ybuf", name="y", bufs=8)
                  if resid_dram is not None:
                      resid = sm.tile([128, D], F32, tag="xin",
                                      name="resid", bufs=3)
                      nc.sync.dma_start(
                          resid[:], resid_dram[b, I * 128:(I + 1) * 128, :])
                      nc.vector.tensor_tensor(y[:], po[:], resid[:],
                                              op=Alu.add)
                  else:
                      nc.vector.tensor_copy(y[:], po[:])
                  ybufs.append(y)
                  st6 = sm.tile([128, 6], F32, tag="st6", name="st6")
                  nc.vector.bn_stats(st6[:], y[:])
                  nc.vector.bn_aggr(mvh[:, 2 * I:2 * I + 2], st6[:])
              # ---- LN apply for this half (gamma/beta folded) ----
              lnv = sm.tile([128, 4], F32, tag="std", name="lnv", bufs=2)
              nc.scalar.activation(lnv[:], mvh[:, 1:8:2], Act.Ln,
                                   bias=epsT[:], scale=1.0)
              rstd = sm.tile([128, 4], F32, tag="rstd", name="rstd", bufs=2)
              nc.scalar.activation(rstd[:], lnv[:], Act.Exp, scale=-0.5)
              for I in range(4):
                  hout = work.tile([128, D], BF16, tag="hblk", bufs=8,
                                   name=f"{out_name}_{b}_{I}")
                  nc.vector.tensor_scalar(
                      hout[:], ybufs[I][:], mvh[:, 2 * I:2 * I + 1],
                      rstd[:, I:I + 1], op0=Alu.subtract, op1=Alu.mult)
                  houts.append(hout)
              return houts

          def transpose_chunks(chunks, out_name):
              out = [work.tile([128, BL * S], BF16, tag="xfrm", bufs=6,
                               name=f"{out_name}_{dc}") for dc in range(2)]
              for pc in range(PC):
                  for dc in range(2):
                      transpose128(out[dc][:, pc * 128:(pc + 1) * 128],
                                   chunks[pc][:, dc * 128:(dc + 1) * 128])
              return out

          # ================= blocks 1, 2 (interleaved) =================
          xT1 = [xT[(1, 0)], xT[(1, 1)]]
          QT1 = projQ(xT1, 'b1_wq', 'b1_qbr', 'QT1')
          V1 = projN(xT1, 'b1_wv', 'b1_vbr', 'V1')
          prep_x(2, x2d)
          xT2 = [xT[(2, 0)], xT[(2, 1)]]
          QT2 = projQ(xT2, 'b2_wq', 'b2_qbr', 'QT2')

          # alpha = softmax_h(x1 . keyh): fill-in work for the ramp-up
          alphas = []
          for pc in range(PC):
              psb = pO.tile([128, D], F32, tag="pO", name="psb")
              for kc in range(2):
                  nc.tensor.matmul(psb[:, 0:H],
                                   xT[(1, kc)][:, pc * 128:(pc + 1) * 128],
                                   C[f"keyhT__{kc}"][:],
                                   start=(kc == 0), stop=(kc == 1))
              ea = sm.tile([128, H], F32, tag="ea", name="ea")
              rsa = sm.tile([128, 1], F32, tag="rsa", name="rsa")
              nc.scalar.activation(ea[:], psb[:, 0:H], Act.Exp,
                                   accum_out=rsa[:])
              ira = sm.tile([128, 1], F32, tag="ira", name="ira")
              nc.vector.reciprocal(ira[:], rsa[:])
              al = sm.tile([128, H], F32, tag="alpha", name="alpha", bufs=8)
              nc.vector.tensor_scalar(al[:], ea[:], ira[:], None,
                                      op0=Alu.mult)
              alphas.append(al)

          st1 = phase1('b1', QT1, 0)
          st1 = phase1('b1', QT1, 1, st=st1)
          tok1 = mk_token([st1['Ep'][H - 1][3][1]])
          phase2_sqrt(st1, 0, token=tok1)
          phase2_sqrt(st1, 1, token=tok1)
          phase2_exp(st1, 0)
          phase2_exp(st1, 1)
          hq0 = phase3('b1', st1, QT1, V1, 0, resid_dram=x1d, out_name='hq')
          st2 = phase1('b2', QT2, 0)
          hq1 = phase3('b1', st1, QT1, V1, 1, resid_dram=x1d, out_name='hq')
          st2 = phase1('b2', QT2, 1, st=st2)
          hq = hq0 + hq1
          hqT = transpose_chunks(hq, 'hqT')
          V2 = projN(xT2, 'b2_wv', 'b2_vbr', 'V2')
          tok2 = mk_token([st2['Ep'][H - 1][3][1], hq1[3]])
          phase2_sqrt(st2, 0, token=tok2)
          phase2_sqrt(st2, 1, token=tok2)
          phase2_exp(st2, 0)
          phase2_exp(st2, 1)
          ha0 = phase3('b2', st2, QT2, V2, 0, resid_dram=x2d, out_name='ha')
          ha1 = phase3('b2', st2, QT2, V2, 1, resid_dram=x2d, out_name='ha')
          ha = ha0 + ha1
          haT = transpose_chunks(ha, 'haT')

          # ================= block 3 =================
          K3T = projT(hqT, 'b3_wk', 'b3_kbT', 'K3T')
          V3 = projN(haT, 'b3_wv', 'b3_vbr', 'V3')
          # c rows -> cTt [1, 4*1024], col (h%4)*1024 + b*512 + i (per grp)
          csbs, csbbs, ecs = [], [], []  # c (f32), c (bf16), exp(c)
          ctparts = {}
          for pc in range(PC):
              psc = p1.tile([128, 1024], F32, tag="p1", name="psc")[:, 0:H]
              for kc in range(2):
                  nc.tensor.matmul(psc[:],
                                   K3T[kc][:, pc * 128:(pc + 1) * 128],
                                   C[f"q03__{kc}"][:],
                                   start=(kc == 0), stop=(kc == 1))
              csb = sm.tile([128, H], F32, tag="csb", name="csb", bufs=8)
              nc.vector.tensor_copy(csb[:], psc[:])
              csbs.append(csb)
              csbb = sm.tile([128, H], BF16, tag="csbb", name="csbb", bufs=8)
              nc.vector.tensor_copy(csbb[:], psc[:])
              csbbs.append(csbb)
              ec = sm.tile([128, H], BF16, tag="ec", name="ec", bufs=8)
              nc.scalar.activation(ec[:], psc[:], Act.Exp)
              ecs.append(ec)
          # Eb3[h][b] [128, 512] = exp(c) broadcast along partitions (the
          # scan input): c^T via PE transpose, ones-broadcast matmul, exp.
          # Stored as halves of the Ep{hp}_3-shaped pair tiles.
          Eb3 = [[None] * BL for _ in range(H)]
          for hp in range(H // 2):
              pair = [work.tile([128, 1024], BF16, tag=f"Ep{hp}_3",
                                bufs=2, name=f"Eb3_{hp}_{b}")
                      for b in range(BL)]
              for sub in range(2):
                  h = 2 * hp + sub
                  ptc = pT.tile([128, 1024], BF16, tag="pT", name="ptc")
                  for pc in range(PC):
                      nc.tensor.transpose(ptc[0:1, pc * 128:(pc + 1) * 128],
                                          csbbs[pc][:, h:h + 1], ident[:])
                  cTh = rot.tile([1, 1024], BF16, tag="cT", bufs=2,
                                 name=f"cT_{h}")
                  nc.vector.tensor_copy(cTh[:], ptc[0:1, 0:1024])
                  ps = p1.tile([128, 1024], F32, tag="p1", name="bc_ps")
                  for b in range(BL):
                      nc.tensor.matmul(
                          ps[:, b * 512:(b + 1) * 512], ones1[0:1, 0:128],
                          cTh[0:1, b * 512:(b + 1) * 512],
                          start=True, stop=True)
                  for b in range(BL):
                      nc.scalar.activation(
                          pair[b][:, sub * 512:(sub + 1) * 512],
                          ps[:, b * 512:(b + 1) * 512], Act.Exp)
                      Eb3[h][b] = pair[b][:, sub * 512:(sub + 1) * 512]
          # rank-1 upper contributions: w[b][J][1, 264] = sum_j exp(c_j) V[j]
          wsb = [[None] * 4 for _ in range(BL)]
          for b in range(BL):
              for J in range(1, 4):
                  pc = b * 4 + J
                  pw = p1.tile([128, 1024], F32, tag="p1",
                               name="pw")[0:1, 0:H * 33]
                  for h in range(H):
                      nc.tensor.matmul(pw[0:1, h * 33:(h + 1) * 33],
                                       ecs[pc][:, h:h + 1],
                                       V3[pc][:, h * 33:(h + 1) * 33],
                                       start=True, stop=True)
                  wt = sm.tile([1, H * 33], BF16, tag="wsb", name="wsb",
                               bufs=6)
                  nc.vector.tensor_copy(wt[:], pw[:])
                  wsb[b][J] = wt

          st3 = phase1('b3', None, 0, Eb3=Eb3)
          st3 = phase1('b3', None, 1, st=st3, Eb3=Eb3)
          tok3 = mk_token([Eb3[H - 1][1], ha1[3]])
          phase2_sqrt(st3, 0, token=tok3)
          phase2_sqrt(st3, 1, token=tok3)
          phase2_exp(st3, 0)
          phase2_exp(st3, 1)
          h30 = phase3('b3', st3, None, V3, 0, csbs=csbs, wsb=wsb,
                       out_name='h3')
          h31 = phase3('b3', st3, None, V3, 1, csbs=csbs, wsb=wsb,
                       out_name='h3')
          h3 = h30 + h31

          # ================= final stage =================
          # per-head transposed h3: h3Tp[grp] [32, 4*1024],
          # col (h%4)*1024 + pc*128
          h3Tp = [rot.tile([32, 4 * 1024], BF16, tag="h3Tp", bufs=2,
                           name=f"h3Tp_{grp}") for grp in range(2)]
          for pc in range(PC):
              for grp in range(2):
                  ptv = pT.tile([128, 1024], BF16, tag="pT", name="ptv")
                  for hh in range(4):
                      h = grp * 4 + hh
                      nc.tensor.transpose(ptv[0:32, hh * 128:(hh + 1) * 128],
                                          h3[pc][:, h * 32:(h + 1) * 32],
                                          ident[:])
                  src3 = ptv[0:32, 0:512].rearrange("p (h c) -> p h c", h=4)
                  dview = h3Tp[grp].rearrange("p (h c) -> p h c", h=4)[
                      0:32, :, pc * 128:pc * 128 + 128]
                  nc.vector.tensor_copy(dview, src3)
          for pc in range(PC):
              b, ic = divmod(pc, 4)
              vhalves = []
              for half in range(2):
                  ps = p1.tile([128, 1024], F32, tag="p1", name="val_ps")
                  for hh in range(4):
                      h = half * 4 + hh
                      seg = ps[:, hh * 256:(hh + 1) * 256]
                      nc.tensor.matmul(
                          seg,
                          h3Tp[half][0:32, hh * 1024 + pc * 128:
                                     hh * 1024 + pc * 128 + 128],
                          C[f"lvw__{h}"][:],
                          start=True, stop=False)
                      nc.tensor.matmul(seg, ones1[0:1, 0:128],
                                       C['lvbr'][0:1, h * 256:(h + 1) * 256],
                                       start=False, stop=True)
                  val = rot.tile([128, 1024], BF16, tag="val",
                                 name="val", bufs=2)
                  nc.scalar.activation(val[:], ps[:], Act.Sigmoid)
                  vhalves.append(val)
              alpha = alphas[pc]
              acc = sm.tile([128, D], F32, tag="acc", name="acc", bufs=2)
              nc.vector.tensor_scalar(acc[:], vhalves[0][:, 0:256],
                                      alpha[:, 0:1], None, op0=Alu.mult)
              for h in range(1, H):
                  half, hh = divmod(h, 4)
                  acc2 = sm.tile([128, D], F32, tag="acc", name="acc2",
                                 bufs=2)
                  nc.vector.scalar_tensor_tensor(
                      acc2[:], vhalves[half][:, hh * 256:(hh + 1) * 256],
                      alpha[:, h:h + 1], acc[:],
                      op0=Alu.mult, op1=Alu.add)
                  acc = acc2
              nc.sync.dma_start(outd[b, ic * 128:(ic + 1) * 128, :], acc[:])

    nc.compile()
    return nc


_GRAPH_CACHE = {}


def _get_graph(consts, g2):
    key = tuple(np.float32(v) for blk in ('b1', 'b2', 'b3')
                for v in g2[blk])
    if key not in _GRAPH_CACHE:
        _GRAPH_CACHE[key] = _build(consts, g2)
    return _GRAPH_CACHE[key]


def kernel(**inputs):
    consts, g2 = _host_prep(inputs)
    nc = _get_graph(consts, g2)
    q = np.ascontiguousarray(np.asarray(inputs['q_emb'], np.float32))
    qa = np.ascontiguousarray(np.asarray(inputs['qa_emb'], np.float32))
    in_maps = []
    for core in range(NCORES):
        m = {'x1': q[core * BL:(core + 1) * BL],
             'x2': qa[core * BL:(core + 1) * BL]}
        m.update(consts)
        in_maps.append(m)
    res = run_bass_kernel_spmd(nc, in_maps, core_ids=list(range(NCORES)))
    out = np.concatenate([res.results[c]['out'] for c in range(NCORES)],
                         axis=0)
    return out.astype(np.float32)


# revision 55
# speedup vs baseline: 7.2044x; 1.1792x over previous
"""Trainium2 Bass kernel for nn_Architecture_11879879540882 (AKT-style
monotonic sparse attention), data-parallel over batch on 8 NeuronCores.

Self-contained: hardcodes shapes B=16,S=512,D=256,H=8,DK=32, shards the batch
2-per-core, runs one Bass graph SPMD via run_bass_kernel_spmd, gathers output.

Algorithm notes (validated vs the jax reference):
 - blocks 1/2: k-projection shares q weights and inputs -> K == Q, so the
   score matrix is SYMMETRIC.
 - masked softmax + cumsum distance statistics collapse into ONE reversed
   masked scan: state=(E+state)*mask -> all suffix sums + masked row total.
 - dist = sqrt(suffix*pos*g^2/r), total = exp(-dist)   [g = -softplus(gamma)]
 - second softmax is UNMASKED (reproduces the reference's non-inplace
   masked_fill bug); the 1e-5 clip on `total` is skipped (~1e-4 error).
 - all LayerNorm gamma/beta fold into downstream weights on host.
 - block3's query is position-independent -> its score rows are broadcasts.

Redesign vs the first working version (2.x speedup targets):
 - TRIANGLE NARROWING: for row-chunk I (128 rows), cols >= (I+1)*128 have
   total==1, so phase-1 exp/scan/stat-write and phase-2 sqrt/exp only cover
   (I+1)*128 cols (62.5% of full).
 - TRANSPOSE-BEFORE-EXP: phase 3 transposes z=s*total (PE), then the ACT
   exp reads the transposed PSUM directly -> P^T lands in SBUF with no
   separate PSUM->SBUF copy instruction.
 - SYMMETRY REUSE: for b1/b2 the strict-upper P^T tiles equal phase-1's
   exp(s) tiles (kept in SBUF); for b3 they are rank-1 (broadcast scores)
   and collapse into tiny [1,x] matmuls + a ones-broadcast matmul.
 - ENGINE REBALANCE: distance-stat writes + att copies + final accumulate
   move to the (otherwise idle) Pool engine; bigA is reused in place for
   sqrt/exp/z (saves ~36KB/partition of SBUF).
"""
import sys
import numpy as np

for _p in ('/opt/trn_rl_repo',):
    if _p not in sys.path:
        sys.path.append(_p)

import ml_dtypes
import concourse.bass as bass
import concourse.bacc as bacc
import concourse.tile as tile
import concourse.mybir as mybir
from concourse.bass_utils import run_bass_kernel_spmd

F32 = mybir.dt.float32
BF16 = mybir.dt.bfloat16
Alu = mybir.AluOpType
Act = mybir.ActivationFunctionType
NPBF = ml_dtypes.bfloat16

B, S, D, H, DK = 16, 512, 256, 8, 32
NCORES = 8
BL = B // NCORES          # local batches per core = 2
PC = BL * 4               # 128-row position chunks per core = 8
LN_EPS = 1e-5

REV = (slice(None), slice(None, None, -1))

# packed-constant layouts: (name, ncols); all [128, ncols] bf16.
PACKA = ([('ident', 128)]
         + [(f'b1_{w}__{k}', 256) for w in ('wq', 'wv', 'wo')
            for k in range(2)]
         + [(f'mpad__{i}', 513) for i in range(4)]
         + [(f'posm__{i}', 512) for i in range(4)])
PACKB = ([(f'b2_{w}__{k}', 256) for w in ('wq', 'wv', 'wo')
          for k in range(2)]
         + [(f'b3_{w}__{k}', 256) for w in ('wk', 'wv', 'wo')
            for k in range(2)]
         + [(f'q03__{k}', 8) for k in range(2)]
         + [(f'keyhT__{k}', 8) for k in range(2)])
PACK32 = [('b3_kbT', 2), ('g2b_b1', 8), ('g2b_b2', 8), ('g2b_b3', 8)]


def _softplus(x):
    return np.logaddexp(0.0, x)


def _host_prep(inp):
    """Parameter preprocessing on host. Returns (consts dict, g2 dict)."""
    p = {k: np.asarray(v, np.float32) for k, v in inp.items()}
    c = {}
    s4 = np.float32(DK ** -0.25)
    bf = lambda x: np.ascontiguousarray(np.asarray(x, np.float32)).astype(NPBF)
    colpack = lambda b: np.ascontiguousarray(
        np.asarray(b, np.float32).reshape(2, 128).T).astype(np.float32)

    for blk in ('b1', 'b2'):
        c[blk + '_wq'] = bf(p[blk + '_qw'] * s4)
        c[blk + '_qbr'] = bf((p[blk + '_qb'] * s4)[None, :])
        c[blk + '_wv'] = bf(p[blk + '_vw'])
        c[blk + '_vbr'] = bf(p[blk + '_vb'][None, :])
        c[blk + '_wo'] = bf(p[blk + '_ow'])
        c[blk + '_obr'] = bf(p[blk + '_ob'][None, :])
    know = p['know'][0, 0]
    q03 = ((know @ p['b3_qw'] + p['b3_qb']) / np.sqrt(DK)).reshape(H, DK)
    Q03 = np.zeros((D, H), np.float32)
    for h in range(H):
        Q03[h * DK:(h + 1) * DK, h] = q03[h]
    c['q03'] = bf(Q03)
    g1, be1 = p['b1_lng'], p['b1_lnb']
    c['b3_wk'] = bf(p['b3_kw'] * g1[:, None])
    c['b3_kbT'] = colpack(p['b3_kb'] + be1 @ p['b3_kw'])
    g2_, be2 = p['b2_lng'], p['b2_lnb']
    c['b3_wv'] = bf(p['b3_vw'] * g2_[:, None])
    c['b3_vbr'] = bf((p['b3_vb'] + be2 @ p['b3_vw'])[None, :])
    c['b3_wo'] = bf(p['b3_ow'])
    c['b3_obr'] = bf((p['b3_ob'] + know)[None, :])
    g3, be3 = p['b3_lng'], p['b3_lnb']
    lvw = np.zeros((H, DK, D), np.float32)
    lvb = np.zeros((H, D), np.float32)
    for h in range(H):
        sl = slice(h * DK, (h + 1) * DK)
        lvw[h] = p['lv_w'] * g3[sl][:, None]
        lvb[h] = p['lv_b'] + be3[sl] @ p['lv_w']
    c['lvw'] = bf(lvw)                            # -> lvw__h [32,256]
    c['lvbr'] = bf(lvb.reshape(1, H * D))         # [1, 2048]
    know_r = know.reshape(H, DK)
    keyh = 1.0 / (1.0 + np.exp(-(know_r @ p['lk_w'] + p['lk_b'])))
    c['keyhT'] = bf(keyh.T)                       # [D, H]

    # padded inclusive mask: mpad[ic][p, j] = (j <= i_p), j in [0, 512];
    # strict mask is the shifted view mpad[:, 1:513].
    i = np.arange(S + 1, dtype=np.int64)
    mpad = np.zeros((4, 128, S + 1), np.float32)
    pos = np.zeros((4, 128, S), np.float32)
    for ic in range(4):
        ii = np.arange(ic * 128, (ic + 1) * 128, dtype=np.int64)[:, None]
        mpad[ic] = (i[None, :] <= ii)
        pos[ic] = np.abs(ii - i[None, :S])
    for blk in ('b1', 'b2', 'b3'):
        g2v = (_softplus(p[blk + '_gam'][:, 0, 0]) ** 2).astype(np.float32)
        c['g2b_' + blk] = np.ascontiguousarray(
            np.broadcast_to(g2v[None, :], (128, H))).astype(np.float32)
    c['mpad'] = bf(mpad)
    c['posm'] = bf(pos)
    c['ident'] = bf(np.eye(128))

    flat = {}
    for name, a in c.items():
        if a.ndim == 2 and a.shape[0] > 128:
            for kc in range(a.shape[0] // 128):
                flat[f"{name}__{kc}"] = np.ascontiguousarray(
                    a[kc * 128:(kc + 1) * 128])
        elif a.ndim == 3:
            for kc in range(a.shape[0]):
                flat[f"{name}__{kc}"] = np.ascontiguousarray(a[kc])
        else:
            flat[name] = a
    # pack the [128, x] bf16 consts into two big arrays (2 DMAs instead of
    # ~30 -- the serial SP DMA-issue time dominated kernel startup).
    packed = {}
    for pname, layout in (('packA', PACKA), ('packB', PACKB)):
        tot = sum(w for _, w in layout)
        arr = np.zeros((128, tot), NPBF)
        off = 0
        for nm, wd in layout:
            arr[:, off:off + wd] = flat.pop(nm)
            off += wd
        packed[pname] = arr
    tot = sum(w for _, w in PACK32)
    arr = np.zeros((128, tot), np.float32)
    off = 0
    for nm, wd in PACK32:
        arr[:, off:off + wd] = flat.pop(nm)
        off += wd
    packed['pack32'] = arr
    flat.update(packed)
    g2 = {blk: [float(v) for v in
                (_softplus(p[blk + '_gam'][:, 0, 0]) ** 2)]
          for blk in ('b1', 'b2', 'b3')}
    return flat, g2


_NPDT = {np.dtype(np.float32): F32, np.dtype(NPBF): BF16}


def _build(consts, g2, reps=1):
    """Builds the per-core Bass graph (BL local batches). reps>1 repeats
    the whole computation on-device (for slope-based timing)."""
    nc = bacc.Bacc("TRN2", target_bir_lowering=False, debug=False)

    x1d = nc.dram_tensor("x1", [BL, S, D], F32, kind="ExternalInput")
    x2d = nc.dram_tensor("x2", [BL, S, D], F32, kind="ExternalInput")
    outd = nc.dram_tensor("out", [BL, S, D], F32, kind="ExternalOutput")
    cd = {name: nc.dram_tensor(name, list(a.shape), _NPDT[a.dtype],
                               kind="ExternalInput")
          for name, a in consts.items()}

    from contextlib import ExitStack
    with tile.TileContext(nc) as tc, ExitStack() as _ps:
        sb = _ps.enter_context(tc.tile_pool(name="const", bufs=1))
        work = _ps.enter_context(tc.tile_pool(name="work", bufs=1))
        sm = _ps.enter_context(tc.tile_pool(name="sm", bufs=4))
        rot = _ps.enter_context(tc.tile_pool(name="rot", bufs=4))
        p1 = _ps.enter_context(tc.tile_pool(name="p1", bufs=2, space="PSUM"))
        pT = _ps.enter_context(tc.tile_pool(name="pT", bufs=2, space="PSUM"))
        pAV = _ps.enter_context(tc.tile_pool(name="pAV", bufs=1, space="PSUM"))
        pO = _ps.enter_context(tc.tile_pool(name="pO", bufs=1, space="PSUM"))

        # ---------- constants (packed: 3 big DMAs + a few stragglers) ----
        C = {}
        for pname, layout, dt in (('packA', PACKA, BF16),
                                  ('packB', PACKB, BF16),
                                  ('pack32', PACK32, F32)):
            tot = sum(wd for _, wd in layout)
            t = sb.tile([128, tot], dt, name="c_" + pname)
            nc.sync.dma_start(t[:], cd[pname][:])
            off = 0
            for nm, wd in layout:
                C[nm] = t[:, off:off + wd]
                off += wd

        def _prio(name):
            for i, k in enumerate(('b1_', 'b2_', 'b3_', 'lvbr', 'lv', 'key')):
                if name.startswith(k):
                    return i
            return 99
        for name in sorted(cd, key=_prio):
            if name in ('packA', 'packB', 'pack32'):
                continue
            ap = cd[name]
            t = sb.tile(list(ap.shape), ap.dtype, name="c_" + name)
            nc.sync.dma_start(t[:], ap[:])
            C[name] = t
        ones1 = sb.tile([1, 512], BF16, name="ones1")
        nc.vector.memset(ones1[:], 1.0)
        ones0 = sb.tile([1, 128], BF16, name="ones0")   # [0,1,1,...]
        nc.vector.memset(ones0[:], 1.0)
        nc.vector.memset(ones0[0:1, 0:1], 0.0)
        epsT = sb.tile([128, 1], F32, name="epsT")
        nc.vector.memset(epsT[:], LN_EPS)
        ident = C['ident']
        mpad = [C[f"mpad__{ic}"] for ic in range(4)]
        posm = [C[f"posm__{ic}"] for ic in range(4)]

        def transpose128(dst, src, cp_engine=None):
            """dst[128,128] SBUF bf16 = src.T via PE + copy."""
            pt = pT.tile([128, 1024], BF16, tag="pT", name="pt_t")
            nc.tensor.transpose(pt[:, 0:128], src, ident[:])
            (cp_engine or nc.vector).tensor_copy(dst, pt[:, 0:128])

        # ---------- input prep: transposed bf16 copies of x1/x2 ----------
        # x DMAs issue from the (idle) Pool queue so they run in parallel
        # with the SP queue's long constants sequence.
        for _rep in range(reps):
          xT = {}

          def prep_x(xi, xd):
              for dc in range(2):
                  xT[(xi, dc)] = work.tile([128, BL * S], BF16, tag="xfrm",
                                           bufs=6, name=f"xT{xi}_{dc}")
              for pc in range(PC):
                  b, ic = divmod(pc, 4)
                  t = sm.tile([128, D], F32, tag="xin", name="xin", bufs=3)
                  nc.sync.dma_start(t[:], xd[b, ic * 128:(ic + 1) * 128, :])
                  tb = sm.tile([128, D], BF16, tag="xbf", name="xbf_t", bufs=2)
                  nc.vector.tensor_copy(tb[:], t[:])
                  for dc in range(2):
                      transpose128(xT[(xi, dc)][:, pc * 128:(pc + 1) * 128],
                                   tb[:, dc * 128:(dc + 1) * 128])

          prep_x(1, x1d)

          # ---------- projections ----------
          def projQ(xTloc, wname, brname, out_name):
              """Head-packed transposed projection QTp [64, H/2*BL*S]:
              head h sits at partition base 32*(h%2),
              cols (h//2)*1024 + b*512 + pos."""
              QTp = work.tile([64, (H // 2) * BL * S], BF16, name=out_name,
                              tag="QTp", bufs=2)
              qbr = C[brname]
              for hp in range(H // 2):
                  for half in range(BL):
                      ps = p1.tile([128, 1024], F32, tag="p1",
                                   name="projQ_ps")
                      for sub in range(2):
                          h = 2 * hp + sub
                          bp = 32 * sub
                          for kc in range(2):
                              nc.tensor.matmul(
                                  ps[bp:bp + 32, 0:512],
                                  C[f"{wname}__{kc}"][:, h * 32:(h + 1) * 32],
                                  xTloc[kc][:, half * 512:(half + 1) * 512],
                                  start=(kc == 0), stop=False)
                          nc.tensor.matmul(ps[bp:bp + 32, 0:512],
                                           qbr[0:1, h * 32:(h + 1) * 32],
                                           ones1[0:1, :], start=False,
                                           stop=True)
                      nc.vector.tensor_copy(
                          QTp[0:64, hp * 1024 + half * 512:
                              hp * 1024 + (half + 1) * 512],
                          ps[0:64, 0:512])
              return QTp

          def projT(xTloc, wname, bTname, out_name):
              """Chunk-transposed projection out[dc][128, BL*S] (for K3T)."""
              out = [work.tile([128, BL * S], BF16, tag="xfrm", bufs=6,
                               name=f"{out_name}_{dc}") for dc in range(2)]
              bT = C[bTname]
              for dc in range(2):
                  for hh in range(BL):
                      ps = p1.tile([128, 1024], F32, tag="p1", name="projT_ps")
                      for kc in range(2):
                          nc.tensor.matmul(
                              ps[:, 0:512],
                              C[f"{wname}__{kc}"][:, dc * 128:(dc + 1) * 128],
                              xTloc[kc][:, hh * 512:(hh + 1) * 512],
                              start=(kc == 0), stop=(kc == 1))
                      nc.scalar.activation(out[dc][:, hh * 512:(hh + 1) * 512],
                                           ps[:, 0:512], Act.Identity,
                                           bias=bT[:, dc:dc + 1], scale=1.0)
              return out

          def projN(xTloc, wname, brname, out_name):
              """Natural projection, head-packed with a ones column:
              out[pc] [128, 8*33]: head h = cols [33h, 33h+32), col 33h+32=1.
              The ones column makes the AV matmul also emit softmax-2 row
              sums."""
              out = [work.tile([128, H * 33], BF16, tag="Vt", bufs=12,
                               name=f"{out_name}_{pc}") for pc in range(PC)]
              br = C[brname]
              for pc in range(PC):
                  ps = pO.tile([128, D], F32, tag="pO", name="projN_ps")
                  for kc in range(2):
                      nc.tensor.matmul(ps[:],
                                       xTloc[kc][:, pc * 128:(pc + 1) * 128],
                                       C[f"{wname}__{kc}"],
                                       start=(kc == 0), stop=False)
                  nc.tensor.matmul(ps[:], ones1[0:1, 0:128], br[:],
                                   start=False, stop=True)
                  ov = out[pc].rearrange("p (h c) -> p h c", c=33)
                  nc.vector.tensor_copy(ov[:, :, 0:32],
                                        ps.rearrange("p (h c) -> p h c",
                                                     c=32))
                  nc.vector.memset(ov[:, :, 32:33], 1.0)
              return out

          # ============== attention block phases ==============
          # state[(b)] = dict with Ep tiles, bigA tiles, etc.

          def qk_ph1(QTp, h, I, b, ps, w):
              """phase-1 scores ps[:, 0:w] for one batch-half."""
              hp, sub = divmod(h, 2)
              bp = 32 * sub
              base = hp * 1024 + b * 512
              nc.tensor.matmul(
                  ps[:, 0:w],
                  QTp[bp:bp + 32, base + I * 128: base + I * 128 + 128],
                  QTp[bp:bp + 32, base: base + w],
                  start=True, stop=True)

          def phase1(blk, QTp, b, st=None, Eb3=None):
              """QK + exp + scans + stat-writes for ONE batch-half b.
              Accumulates into/returns state st: Ep[h][I][b], bigA[b][I]."""
              strict = (blk == 'b3')
              g2b = C['g2b_' + blk]
              if st is None:
                  st = {'Ep': [[[None] * BL for _ in range(4)]
                               for _ in range(H)],
                        'bigA': [[None] * 4 for _ in range(BL)],
                        'strict': strict}
              Ep, bigA = st['Ep'], st['bigA']
              for I in range(4):
                  w = (I + 1) * 128
                  bigA[b][I] = work.tile([128, H * w], BF16,
                                         tag=f"bigA{I}", bufs=2,
                                         name=f"bigA_{blk}_{b}_{I}")
              if strict:
                  # Eb3[h][b] = exp(broadcast score row), reused for all I.
                  for h in range(H):
                      for I in range(4):
                          Ep[h][I][b] = Eb3[h][b]
              for I in range(4):
                  w = (I + 1) * 128
                  maskv = (mpad[I][:, 1:w + 1] if strict
                           else mpad[I][:, 0:w])
                  r8 = sm.tile([128, H], F32, tag="r8", name="r8", bufs=2)
                  Rvs = {}
                  if not strict:
                      for hp in range(H // 2):
                          ps = p1.tile([128, 1024], F32, tag="p1",
                                       name="qk1_ps")
                          for sub in range(2):
                              h = 2 * hp + sub
                              bp = 32 * sub
                              base = hp * 1024 + b * 512
                              nc.tensor.matmul(
                                  ps[:, sub * 512: sub * 512 + w],
                                  QTp[bp:bp + 32,
                                      base + I * 128: base + I * 128 + 128],
                                  QTp[bp:bp + 32, base: base + w],
                                  start=True, stop=True)
                          e = work.tile([128, 2 * w], BF16,
                                        tag=f"Ep{hp}_{I}", bufs=2,
                                        name=f"Ep_{hp}_{I}_{b}")
                          nc.scalar.activation(
                              e.rearrange("p (s c) -> p s c", c=w),
                              ps.rearrange("p (s c) -> p s c",
                                           c=512)[:, :, 0:w], Act.Exp)
                          Ep[2 * hp][I][b] = e[:, 0:w]
                          Ep[2 * hp + 1][I][b] = e[:, w:2 * w]
                  for h in range(H):
                      Rv = sm.tile([128, 512], BF16, tag="Rv",
                                   name="Rv", bufs=8)
                      nc.vector.tensor_tensor_scan(
                          Rv[:, 0:w][REV], Ep[h][I][b][:, 0:w][REV],
                          maskv[REV], 0.0, op0=Alu.add, op1=Alu.mult)
                      nc.vector.tensor_copy(r8[:, h:h + 1], Rv[:, 0:1])
                      Rvs[h] = Rv
                  r8m = sm.tile([128, H], F32, tag="r8m", name="r8m", bufs=2)
                  nc.vector.tensor_scalar(r8m[:], r8[:], 1e-30, None,
                                          op0=Alu.max)
                  rc = sm.tile([128, H], F32, tag="rc", name="rc", bufs=2)
                  nc.vector.reciprocal(rc[:], r8m[:])
                  rgb = sm.tile([128, H], F32, tag="rgb", name="rgb", bufs=2)
                  nc.vector.tensor_tensor(rgb[:], rc[:], g2b[:], op=Alu.mult)
                  bA = bigA[b][I]
                  for h in range(H):
                      nc.vector.scalar_tensor_tensor(
                          bA[:, h * w: h * w + w - 1],
                          posm[I][:, 0:w - 1], rgb[:, h:h + 1],
                          Rvs[h][:, 1:w],
                          op0=Alu.mult, op1=Alu.mult)
                  # col w-1 of every head segment: suffix beyond row end
                  # is empty -> bigA = 0 (gives total == 1 post-exp).
                  bAv = bA.rearrange("p (h c) -> p h c", c=w)
                  nc.vector.memset(bAv[:, :, w - 1:w], 0.0)
              return st

          def mk_token(srcs):
              """[128,1] zero tile data-dependent on srcs (ACT-order fence:
              used as a sqrt bias so the sqrt batch can't start, and hence
              can't interleave table-thrashing, before srcs complete)."""
              tok = sm.tile([128, 1], F32, tag="tok", name="tok", bufs=2)
              if len(srcs) == 1:
                  nc.vector.tensor_scalar(tok[:], srcs[0][:, 0:1], 0.0, None,
                                          op0=Alu.mult)
              else:
                  nc.vector.scalar_tensor_tensor(
                      tok[:], srcs[0][:, 0:1], 0.0, srcs[1][:, 0:1],
                      op0=Alu.mult, op1=Alu.mult)
              return tok

          def phase2_sqrt(st, b, token=None):
              for I in range(4):
                  bA = st['bigA'][b][I]
                  if token is not None:
                      nc.scalar.activation(bA[:], bA[:], Act.Sqrt,
                                           bias=token[:, 0:1], scale=1.0)
                  else:
                      nc.scalar.activation(bA[:], bA[:], Act.Sqrt)

          def phase2_exp(st, b):
              for I in range(4):
                  bA = st['bigA'][b][I]
                  nc.scalar.activation(bA[:], bA[:], Act.Exp, scale=-1.0)

          def phase3(blk, st, QTp, V, b, csbs=None, wsb=None,
                     resid_dram=None, out_name="hout"):
              """z -> transpose -> exp(P^T) -> AV -> Wo -> +resid -> LN stats
              for one batch-half. Returns 4 hout tiles.

              b1/b2: z = s*t in place on bigA, then transpose+exp.
              b3: scores are broadcast rows c_j, so z^T = c_j * t^T with c_j
              PER-PARTITION after the transpose -> fold into the exp's scale
              operand (no z multiply, no broadcast-score tiles at all)."""
              strict = st['strict']
              Ep = st['Ep']
              Wo = [C[blk + '_wo__0'], C[blk + '_wo__1']]
              obr = C[blk + '_obr']
              houts = []
              mvh = sm.tile([128, 8], F32, tag="mvh", name="mvh", bufs=2)
              ybufs = []
              for I in range(4):
                  w = (I + 1) * 128
                  bA = st['bigA'][b][I]
                  # ---- z = s * total, in place on bigA (b1/b2 only) ----
                  if not strict:
                      for hp in range(4):
                          ps3 = p1.tile([128, 1024], F32, tag="p1",
                                        name="qk3_ps")
                          for sub in range(2):
                              h = 2 * hp + sub
                              hq, s2 = divmod(h, 2)
                              bp = 32 * s2
                              base = hq * 1024 + b * 512
                              nc.tensor.matmul(
                                  ps3[:, sub * 512: sub * 512 + w],
                                  QTp[bp:bp + 32,
                                      base + I * 128: base + I * 128 + 128],
                                  QTp[bp:bp + 32, base: base + w],
                                  start=True, stop=True)
                          zv = bA[:, 2 * hp * w: (2 * hp + 2) * w].rearrange(
                              "p (h c) -> p h c", c=w)
                          psv = ps3.rearrange("p (h c) -> p h c",
                                              c=512)[:, :, 0:w]
                          nc.vector.tensor_tensor(zv, psv, zv, op=Alu.mult)
                  # ---- transposed P^T tiles + AV accumulation ----
                  pav = pAV.tile([128, H * 33], F32, tag="pAV", name="pav")
                  Tslabs = []
                  for J in range(I + 1):
                      pTt = pT.tile([128, 1024], BF16, tag="pT", name="pTt")
                      for h in range(H):
                          nc.tensor.transpose(
                              pTt[:, h * 128:(h + 1) * 128],
                              bA[:, h * w + J * 128: h * w + J * 128 + 128],
                              ident[:])
                      Ts = sm.tile([128, 1024], BF16, tag="Tslab",
                                   name="Ts", bufs=4)
                      if not strict:
                          nc.scalar.activation(Ts[:], pTt[:], Act.Exp)
                      else:
                          # z^T = c_j * t^T: c is per-partition post-transpose
                          # but differs per head -> per-head DVE scalar mults
                          # into Ts, then ONE batched in-place exp.
                          csb = csbs[b * 4 + J]
                          for h in range(H):
                              nc.vector.tensor_scalar(
                                  Ts[:, h * 128:(h + 1) * 128],
                                  pTt[:, h * 128:(h + 1) * 128],
                                  csb[:, h:h + 1], None, op0=Alu.mult)
                          nc.scalar.activation(Ts[:], Ts[:], Act.Exp)
                      if strict and I == 0:
                          # zero_pad: P[i=0, :] = 0  (col i=0 of P^T tiles)
                          Tv = Ts.rearrange("p (h c) -> p h c", c=128)
                          nc.vector.memset(Tv[:, :, 0:1], 0.0)
                      Tslabs.append(Ts)
                  orow = ones0 if I == 0 else ones1
                  for h in range(H):
                      hs = slice(h * 33, (h + 1) * 33)
                      for J in range(I + 1):
                          nc.tensor.matmul(
                              pav[:, hs], Tslabs[J][:, h * 128:(h + 1) * 128],
                              V[b * 4 + J][:, hs],
                              start=(J == 0), stop=(J == 3))
                      for J in range(I + 1, 4):
                          if not strict:
                              nc.tensor.matmul(
                                  pav[:, hs],
                                  Ep[h][J][b][:, I * 128:(I + 1) * 128],
                                  V[b * 4 + J][:, hs],
                                  start=False, stop=(J == 3))
                          else:
                              nc.tensor.matmul(
                                  pav[:, hs], orow[0:1, 0:128],
                                  wsb[b][J][0:1, hs],
                                  start=False, stop=(J == 3))
                  # ---- softmax-2 normalize + Wo + resid + LN stats ----
                  pav3 = pav.rearrange("p (h c) -> p h c", c=33)
                  rsm = sm.tile([128, H], F32, tag="rsm", name="rsm")
                  nc.vector.tensor_scalar(rsm[:], pav3[:, :, 32:33],
                                          1e-30, None, op0=Alu.max)
                  rsi = sm.tile([128, H], F32, tag="rsi", name="rsi")
                  nc.vector.reciprocal(rsi[:], rsm[:])
                  att_sb = sm.tile([128, 256], BF16, tag="att",
                                   name="att", bufs=2)
                  nc.vector.tensor_copy(
                      att_sb.rearrange("p (h c) -> p h c", c=32),
                      pav3[:, :, 0:32])
                  att_n = sm.tile([128, 256], BF16, tag="attn",
                                  name="attn", bufs=2)
                  for h in range(H):
                      nc.vector.tensor_scalar(
                          att_n[:, h * 32:(h + 1) * 32],
                          att_sb[:, h * 32:(h + 1) * 32],
                          rsi[:, h:h + 1], None, op0=Alu.mult)
                  attT = [sm.tile([128, 128], BF16, tag=f"attT{i}",
                                  name="attT", bufs=2) for i in range(2)]
                  for dc in range(2):
                      transpose128(attT[dc][:],
                                   att_n[:, dc * 128:(dc + 1) * 128])
                  po = pO.tile([128, D], F32, tag="pO", name="po")
                  nc.tensor.matmul(po[:], attT[0][:], Wo[0][:],
                                   start=True, stop=False)
                  nc.tensor.matmul(po[:], attT[1][:], Wo[1][:],
                                   start=False, stop=False)
                  nc.tensor.matmul(po[:], ones1[0:1, 0:128], obr[:],
                                   start=False, stop=True)
                  y = sm.tile([128, D], BF16, tag="ybuf", name="y", bufs=8)
                  if resid_dram is not None:
                      resid = sm.tile([128, D], F32, tag="xin",
                                      name="resid", bufs=3)
                      nc.sync.dma_start(
                          resid[:], resid_dram[b, I * 128:(I + 1) * 128, :])
                      nc.vector.tensor_tensor(y[:], po[:], resid[:],
                                              op=Alu.add)
                  else:
                      nc.vector.tensor_copy(y[:], po[:])
                  ybufs.append(y)
                  st6 = sm.tile([128, 6], F32, tag="st6", name="st6")
                  nc.vector.bn_stats(st6[:], y[:])
                  nc.vector.bn_aggr(mvh[:, 2 * I:2 * I + 2], st6[:])
              # ---- LN apply for this half (gamma/beta folded) ----
              lnv = sm.tile([128, 4], F32, tag="std", name="lnv", bufs=2)
              nc.scalar.activation(lnv[:], mvh[:, 1:8:2], Act.Ln,
                                   bias=epsT[:], scale=1.0)
              rstd = sm.tile([128, 4], F32, tag="rstd", name="rstd", bufs=2)
              nc.scalar.activation(rstd[:], lnv[:], Act.Exp, scale=-0.5)
              for I in range(4):
                  hout = work.tile([128, D], BF16, tag="hblk", bufs=8,
                                   name=f"{out_name}_{b}_{I}")
                  nc.vector.tensor_scalar(
                      hout[:], ybufs[I][:], mvh[:, 2 * I:2 * I + 1],
                      rstd[:, I:I + 1], op0=Alu.subtract, op1=Alu.mult)
                  houts.append(hout)
              return houts

          def transpose_chunks(chunks, out_name):
              out = [work.tile([128, BL * S], BF16, tag="xfrm", bufs=6,
                               name=f"{out_name}_{dc}") for dc in range(2)]
              for pc in range(PC):
                  for dc in range(2):
                      transpose128(out[dc][:, pc * 128:(pc + 1) * 128],
                                   chunks[pc][:, dc * 128:(dc + 1) * 128])
              return out

          # ================= blocks 1, 2 (interleaved) =================
          xT1 = [xT[(1, 0)], xT[(1, 1)]]
          QT1 = projQ(xT1, 'b1_wq', 'b1_qbr', 'QT1')
          V1 = projN(xT1, 'b1_wv', 'b1_vbr', 'V1')
          prep_x(2, x2d)
          xT2 = [xT[(2, 0)], xT[(2, 1)]]
          QT2 = projQ(xT2, 'b2_wq', 'b2_qbr', 'QT2')

          # alpha = softmax_h(x1 . keyh): fill-in work for the ramp-up
          alphas = []
          for pc in range(PC):
              psb = pO.tile([128, D], F32, tag="pO", name="psb")
              for kc in range(2):
                  nc.tensor.matmul(psb[:, 0:H],
                                   xT[(1, kc)][:, pc * 128:(pc + 1) * 128],
                                   C[f"keyhT__{kc}"][:],
                                   start=(kc == 0), stop=(kc == 1))
              ea = sm.tile([128, H], F32, tag="ea", name="ea")
              rsa = sm.tile([128, 1], F32, tag="rsa", name="rsa")
              nc.scalar.activation(ea[:], psb[:, 0:H], Act.Exp,
                                   accum_out=rsa[:])
              ira = sm.tile([128, 1], F32, tag="ira", name="ira")
              nc.vector.reciprocal(ira[:], rsa[:])
              al = sm.tile([128, H], F32, tag="alpha", name="alpha", bufs=8)
              nc.vector.tensor_scalar(al[:], ea[:], ira[:], None,
                                      op0=Alu.mult)
              alphas.append(al)

          st1 = phase1('b1', QT1, 0)
          st1 = phase1('b1', QT1, 1, st=st1)
          tok1 = mk_token([st1['Ep'][H - 1][3][1]])
          phase2_sqrt(st1, 0, token=tok1)
          phase2_sqrt(st1, 1, token=tok1)
          phase2_exp(st1, 0)
          phase2_exp(st1, 1)
          hq0 = phase3('b1', st1, QT1, V1, 0, resid_dram=x1d, out_name='hq')
          st2 = phase1('b2', QT2, 0)
          hq1 = phase3('b1', st1, QT1, V1, 1, resid_dram=x1d, out_name='hq')
          st2 = phase1('b2', QT2, 1, st=st2)
          hq = hq0 + hq1
          hqT = transpose_chunks(hq, 'hqT')
          V2 = projN(xT2, 'b2_wv', 'b2_vbr', 'V2')
          tok2 = mk_token([st2['Ep'][H - 1][3][1], hq1[3]])
          phase2_sqrt(st2, 0, token=tok2)
          phase2_sqrt(st2, 1, token=tok2)
          phase2_exp(st2, 0)
          phase2_exp(st2, 1)
          ha0 = phase3('b2', st2, QT2, V2, 0, resid_dram=x2d, out_name='ha')
          ha1 = phase3('b2', st2, QT2, V2, 1, resid_dram=x2d, out_name='ha')
          ha = ha0 + ha1
          haT = transpose_chunks(ha, 'haT')

          # ================= block 3 =================
          K3T = projT(hqT, 'b3_wk', 'b3_kbT', 'K3T')
          V3 = projN(haT, 'b3_wv', 'b3_vbr', 'V3')
          # c rows -> cTt [1, 4*1024], col (h%4)*1024 + b*512 + i (per grp)
          csbs, csbbs, ecs = [], [], []  # c (f32), c (bf16), exp(c)
          ctparts = {}
          for pc in range(PC):
              psc = p1.tile([128, 1024], F32, tag="p1", name="psc")[:, 0:H]
              for kc in range(2):
                  nc.tensor.matmul(psc[:],
                                   K3T[kc][:, pc * 128:(pc + 1) * 128],
                                   C[f"q03__{kc}"][:],
                                   start=(kc == 0), stop=(kc == 1))
              csb = sm.tile([128, H], F32, tag="csb", name="csb", bufs=8)
              nc.vector.tensor_copy(csb[:], psc[:])
              csbs.append(csb)
              csbb = sm.tile([128, H], BF16, tag="csbb", name="csbb", bufs=8)
              nc.vector.tensor_copy(csbb[:], psc[:])
              csbbs.append(csbb)
              ec = sm.tile([128, H], BF16, tag="ec", name="ec", bufs=8)
              nc.scalar.activation(ec[:], psc[:], Act.Exp)
              ecs.append(ec)
          # Eb3[h][b] [128, 512] = exp(c) broadcast along partitions (the
          # scan input): c^T via PE transpose, ones-broadcast matmul, exp.
          # Stored as halves of the Ep{hp}_3-shaped pair tiles.
          Eb3 = [[None] * BL for _ in range(H)]
          for hp in range(H // 2):
              pair = [work.tile([128, 1024], BF16, tag=f"Ep{hp}_3",
                                bufs=2, name=f"Eb3_{hp}_{b}")
                      for b in range(BL)]
              for sub in range(2):
                  h = 2 * hp + sub
                  ptc = pT.tile([128, 1024], BF16, tag="pT", name="ptc")
                  for pc in range(PC):
                      nc.tensor.transpose(ptc[0:1, pc * 128:(pc + 1) * 128],
                                          csbbs[pc][:, h:h + 1], ident[:])
                  cTh = rot.tile([1, 1024], BF16, tag="cT", bufs=2,
                                 name=f"cT_{h}")
                  nc.vector.tensor_copy(cTh[:], ptc[0:1, 0:1024])
                  ps = p1.tile([128, 1024], F32, tag="p1", name="bc_ps")
                  for b in range(BL):
                      nc.tensor.matmul(
                          ps[:, b * 512:(b + 1) * 512], ones1[0:1, 0:128],
                          cTh[0:1, b * 512:(b + 1) * 512],
                          start=True, stop=True)
                  for b in range(BL):
                      nc.scalar.activation(
                          pair[b][:, sub * 512:(sub + 1) * 512],
                          ps[:, b * 512:(b + 1) * 512], Act.Exp)
                      Eb3[h][b] = pair[b][:, sub * 512:(sub + 1) * 512]
          # rank-1 upper contributions: w[b][J][1, 264] = sum_j exp(c_j) V[j]
          wsb = [[None] * 4 for _ in range(BL)]
          for b in range(BL):
              for J in range(1, 4):
                  pc = b * 4 + J
                  pw = p1.tile([128, 1024], F32, tag="p1",
                               name="pw")[0:1, 0:H * 33]
                  for h in range(H):
                      nc.tensor.matmul(pw[0:1, h * 33:(h + 1) * 33],
                                       ecs[pc][:, h:h + 1],
                                       V3[pc][:, h * 33:(h + 1) * 33],
                                       start=True, stop=True)
                  wt = sm.tile([1, H * 33], BF16, tag="wsb", name="wsb",
                               bufs=6)
                  nc.vector.tensor_copy(wt[:], pw[:])
                  wsb[b][J] = wt

          st3 = phase1('b3', None, 0, Eb3=Eb3)
          st3 = phase1('b3', None, 1, st=st3, Eb3=Eb3)
          tok3 = mk_token([Eb3[H - 1][1], ha1[3]])
          phase2_sqrt(st3, 0, token=tok3)
          phase2_sqrt(st3, 1, token=tok3)
          phase2_exp(st3, 0)
          phase2_exp(st3, 1)
          h30 = phase3('b3', st3, None, V3, 0, csbs=csbs, wsb=wsb,
                       out_name='h3')
          h31 = phase3('b3', st3, None, V3, 1, csbs=csbs, wsb=wsb,
                       out_name='h3')
          h3 = h30 + h31

          # ================= final stage =================
          # per-head transposed h3: h3Tp[grp] [32, 4*1024],
          # col (h%4)*1024 + pc*128
          h3Tp = [rot.tile([32, 4 * 1024], BF16, tag="h3Tp", bufs=2,
                           name=f"h3Tp_{grp}") for grp in range(2)]
          for pc in range(PC):
              for grp in range(2):
                  ptv = pT.tile([128, 1024], BF16, tag="pT", name="ptv")
                  for hh in range(4):
                      h = grp * 4 + hh
                      nc.tensor.transpose(ptv[0:32, hh * 128:(hh + 1) * 128],
                                          h3[pc][:, h * 32:(h + 1) * 32],
                                          ident[:])
                  src3 = ptv[0:32, 0:512].rearrange("p (h c) -> p h c", h=4)
                  dview = h3Tp[grp].rearrange("p (h c) -> p h c", h=4)[
                      0:32, :, pc * 128:pc * 128 + 128]
                  nc.vector.tensor_copy(dview, src3)
          for pc in range(PC):
              b, ic = divmod(pc, 4)
              vhalves = []
              for half in range(2):
                  ps = p1.tile([128, 1024], F32, tag="p1", name="val_ps")
                  for hh in range(4):
                      h = half * 4 + hh
                      seg = ps[:, hh * 256:(hh + 1) * 256]
                      nc.tensor.matmul(
                          seg,
                          h3Tp[half][0:32, hh * 1024 + pc * 128:
                                     hh * 1024 + pc * 128 + 128],
                          C[f"lvw__{h}"][:],
                          start=True, stop=False)
                      nc.tensor.matmul(seg, ones1[0:1, 0:128],
                                       C['lvbr'][0:1, h * 256:(h + 1) * 256],
                                       start=False, stop=True)
                  val = rot.tile([128, 1024], BF16, tag="val",
                                 name="val", bufs=2)
                  nc.scalar.activation(val[:], ps[:], Act.Sigmoid)
                  vhalves.append(val)
              alpha = alphas[pc]
              acc = sm.tile([128, D], F32, tag="acc", name="acc", bufs=2)
              nc.vector.tensor_scalar(acc[:], vhalves[0][:, 0:256],
                                      alpha[:, 0:1], None, op0=Alu.mult)
              for h in range(1, H):
                  half, hh = divmod(h, 4)
                  acc2 = sm.tile([128, D], F32, tag="acc", name="acc2",
                                 bufs=2)
                  nc.vector.scalar_tensor_tensor(
                      acc2[:], vhalves[half][:, hh * 256:(hh + 1) * 256],
                      alpha[:, h:h + 1], acc[:],
                      op0=Alu.mult, op1=Alu.add)
                  acc = acc2
              nc.sync.dma_start(outd[b, ic * 128:(ic + 1) * 128, :], acc[:])

    nc.compile()
    return nc


_GRAPH_CACHE = {}


def _get_graph(consts, g2):
    key = tuple(np.float32(v) for blk in ('b1', 'b2', 'b3')
                for v in g2[blk])
    if key not in _GRAPH_CACHE:
        _GRAPH_CACHE[key] = _build(consts, g2)
    return _GRAPH_CACHE[key]


def kernel(**inputs):
    consts, g2 = _host_prep(inputs)
    nc = _get_graph(consts, g2)
    q = np.ascontiguousarray(np.asarray(inputs['q_emb'], np.float32))
    qa = np.ascontiguousarray(np.asarray(inputs['qa_emb'], np.float32))
    in_maps = []
    for core in range(NCORES):
        m = {'x1': q[core * BL:(core + 1) * BL],
             'x2': qa[core * BL:(core + 1) * BL]}
        m.update(consts)
        in_maps.append(m)
    res = run_bass_kernel_spmd(nc, in_maps, core_ids=list(range(NCORES)))
    out = np.concatenate([res.results[c]['out'] for c in range(NCORES)],
                         axis=0)
    return out.astype(np.float32)
